# revision 1
# baseline (speedup 1.0000x reference)
"""Teacher-forced decoder LSTM on 8 TRN2 NeuronCores.

Problem: B=256, T=32, V=10000, E=H=512 (fp32).
  step s in 0..30: x = embed[caps[:, s]]
                   gates = x@W_ih.T + h@W_hh.T + b     (i,f,g,o)
                   c = sig(f)*c + sig(i)*tanh(g); h = sig(o)*tanh(c)
                   out[s+1] = h@W_lin.T + b_lin
  out[0] = 0.  Output [T, B, V].

Sharding: data-parallel over batch, B_local=32 per core. Each core:
  phase 1a: gather X = embed[tok] [992, 512], PE-transpose to X.T
  phase 1b: Gx = X@W_ihT + bias (one GEMM, fp32r), stored [8][128, 2048]
  phase 2 (recurrent): per step, 16 M=32 matmuls (h.T stationary) + 4
     selector-matmuls injecting Gx into PSUM; ACT sigmoid/tanh; DVE cell;
     PE transposes h back to [128, 32] chunks stored into h_allT.
  phase 3: logits = h_all@W_linT + b_lin as one [992 x 512 x 10000] GEMM
     (fp32r, W_linT streamed per 2000-col super-chunk), DMA to DRAM.

All matmuls use dtype float32r: full fp32 data, ~1 cycle/row for N>=256,
measured rel err ~1.5e-4 (vs 4e-3 for bf16).
"""
import numpy as np

B_FULL, T, V, E, H = 256, 32, 10000, 512, 512
NCORES = 8
BL = B_FULL // NCORES          # 32 batch per core
S = T - 1                      # 31 recurrent steps
M_TOK = S * BL                 # 992 token rows per core
G4 = 4 * H                     # 2048 gate dims
NSUP = 5                       # vocab super-chunks of 2000
VSUP = V // NSUP               # 2000
NMT = (M_TOK + 127) // 128     # 8 token m-tiles (last is 96 rows)

_CACHE = {}


def _build():
    import concourse.bacc as bacc
    import concourse.mybir as mybir
    from concourse.tile import TileContext
    import concourse.bass as bass

    f32 = mybir.dt.float32
    f32r = mybir.dt.float32r
    i32 = mybir.dt.int32
    SIG = mybir.ActivationFunctionType.Sigmoid
    TANH = mybir.ActivationFunctionType.Tanh
    ADD = mybir.AluOpType.add
    MUL = mybir.AluOpType.mult

    nc = bacc.Bacc()

    emb_d = nc.dram_tensor("emb", [V, E], f32r, kind="ExternalInput")
    wihT_d = nc.dram_tensor("wihT", [E, G4], f32r, kind="ExternalInput")
    whhT_d = nc.dram_tensor("whhT", [H, G4], f32r, kind="ExternalInput")
    biasb_d = nc.dram_tensor("biasb", [128, G4], f32r, kind="ExternalInput")
    wlinT_d = nc.dram_tensor("wlinT", [H, V], f32r, kind="ExternalInput")
    blinb_d = nc.dram_tensor("blinb", [128, V], f32r, kind="ExternalInput")
    tok_d = nc.dram_tensor("tok", [128, NMT], i32, kind="ExternalInput")
    lat_d = nc.dram_tensor("lat", [BL, H], f32r, kind="ExternalInput")
    sel_d = nc.dram_tensor("sel", [128, 4 * BL], f32r, kind="ExternalInput")  # 4 selector mats [128, 32]
    id128_d = nc.dram_tensor("id128", [128, 128], f32r, kind="ExternalInput")
    out_d = nc.dram_tensor("out", [M_TOK, V], f32, kind="ExternalOutput")

    with TileContext(nc) as tc:
        with tc.tile_pool(name="const", bufs=1) as cp, \
             tc.tile_pool(name="state", bufs=1) as st:

            # ---------- constants / state ----------
            sel_sb = cp.tile([128, 4 * BL], f32r, tag="sel_sb")
            nc.sync.dma_start(out=sel_sb[:], in_=sel_d[:])
            id128 = cp.tile([128, 128], f32r, tag="id128")
            nc.sync.dma_start(out=id128[:], in_=id128_d[:])
            tok_sb = cp.tile([128, NMT], i32, tag="tok_sb")
            nc.sync.dma_start(out=tok_sb[:], in_=tok_d[:])
            lat_sb = cp.tile([BL, H], f32r, tag="lat_sb")
            nc.sync.dma_start(out=lat_sb[:], in_=lat_d[:])
            # h_allT[k]: [128, 992] columns are h_{s+1} for step s block
            h_allT = [st.tile([128, M_TOK], f32r, tag=f"h_allT{k}", name=f"h_allT{k}")
                      for k in range(4)]
            hT0 = st.tile([128, 4 * BL], f32r, tag="hT0")      # transposed h0 (latent)
            c_sb = st.tile([BL, H], f32, tag="c_sb")
            nc.vector.memset(c_sb[:], 0.0)
            act_sb = st.tile([BL, G4], f32, tag="act_sb")       # sig/tanh of gates
            t1_sb = st.tile([BL, H], f32, tag="t1_sb")
            t2_sb = st.tile([BL, H], f32, tag="t2_sb")
            th_sb = st.tile([BL, H], f32, tag="th_sb")
            h_sb = st.tile([BL, H], f32r, tag="h_sb")
            # gx + whhT + transpose-psum live until the recurrence ends
            V0 = 512
            p3a = tc.alloc_tile_pool(name="p3a", bufs=1)
            p3aps = tc.alloc_tile_pool(name="p3aps", bufs=1, space="PSUM")
            p3ast = tc.alloc_tile_pool(name="p3ast", bufs=2)
            wl0 = p3a.tile([128, 4 * 512], f32r, tag="wl0")
            for k in range(4):
                nc.sync.dma_start(out=wl0[:, 512 * k:512 * (k + 1)],
                                  in_=wlinT_d[128 * k:128 * (k + 1), 0:512])
            blin0 = p3a.tile([128, 512], f32r, tag="blin0")
            nc.sync.dma_start(out=blin0[:], in_=blinb_d[:, 0:512])
            gxp = tc.alloc_tile_pool(name="gxp", bufs=1)
            tps = tc.alloc_tile_pool(name="tpsum", bufs=1, space="PSUM")
            whhT = gxp.tile([128, 4 * G4], f32r, tag="whhT")
            nc.sync.dma_start(out=whhT[:].rearrange("p (k m) -> p k m", k=4),
                              in_=whhT_d.rearrange("(k p) m -> p k m", k=4))
            gx_tiles = [gxp.tile([128, G4], f32r, tag=f"gx{m}", name=f"gx{m}")
                        for m in range(NMT)]
            # last m-tile has only 96 valid rows; zero the tail once so the
            # full-K selector matmuls never read uninitialized partitions
            # (memset is invalid for f32r, so zero an f32 scratch and cast-copy)
            nc.vector.memset(act_sb[:], 0.0)
            nc.vector.tensor_copy(out=gx_tiles[NMT - 1][96:128, :], in_=act_sb[0:32, :])

            # transpose h0 = latent -> hT0 chunks
            for k in range(4):
                pt = tps.tile([128, 128], f32r, tag="pt", bufs=2)
                nc.tensor.transpose(out=pt[0:128, 0:BL], in_=lat_sb[:, 128 * k:128 * (k + 1)],
                                    identity=id128[0:BL, 0:BL])
                nc.vector.tensor_copy(out=hT0[:, BL * k:BL * (k + 1)], in_=pt[0:128, 0:BL])

            # ---------- phase 1a/1b: gather X, transpose, Gx GEMM ----------
            with tc.tile_pool(name="p1", bufs=1) as p1, \
                 tc.tile_pool(name="p1ps", bufs=1, space="PSUM") as p1ps:
                wihT = p1.tile([128, 4 * G4], f32r, tag="wihT")
                nc.sync.dma_start(out=wihT[:].rearrange("p (k m) -> p k m", k=4),
                                  in_=wihT_d.rearrange("(k p) m -> p k m", k=4))
                biasb = p1.tile([128, G4], f32r, tag="biasb")
                nc.sync.dma_start(out=biasb[:], in_=biasb_d[:])

                for m in range(NMT):
                    rows = min(128, M_TOK - 128 * m)
                    x_m = p1.tile([128, E], f32r, tag="x_m", bufs=1, name=f"x_m{m}")
                    nc.gpsimd.indirect_dma_start(
                        out=x_m[0:rows, :], out_offset=None, in_=emb_d[:],
                        in_offset=bass.IndirectOffsetOnAxis(ap=tok_sb[0:rows, m:m + 1], axis=0))
                    xt_m = []
                    for k in range(4):
                        pt = tps.tile([128, 128], f32r, tag="pt", bufs=2)
                        nc.tensor.transpose(out=pt[0:128, 0:rows],
                                            in_=x_m[0:rows, 128 * k:128 * (k + 1)],
                                            identity=id128[0:rows, 0:rows])
                        xt = p1.tile([128, 128], f32r, tag=f"xtk{k}", bufs=2, name=f"xt{m}_{k}")
                        nc.vector.tensor_copy(out=xt[:, 0:rows], in_=pt[0:128, 0:rows])
                        xt_m.append(xt)
                    pg = p1ps.tile([128, G4], f32, tag="pg")
                    for n in range(4):
                        for k in range(4):
                            nc.tensor.matmul(
                                out=pg[0:rows, 512 * n:512 * (n + 1)],
                                lhsT=xt_m[k][:, 0:rows],
                                rhs=wihT[:, G4 * k + 512 * n: G4 * k + 512 * (n + 1)],
                                start=(k == 0), stop=(k == 3))
                    nc.vector.tensor_tensor(out=gx_tiles[m][0:rows, :], in0=pg[0:rows, :],
                                            in1=biasb[0:rows, :], op=ADD)

            # ---------- phase 2: recurrence, with vocab cols 0..1024 of the
            # logits GEMM interleaved to fill PE gaps and keep the clock warm


            def emit_super0(m):
                rows = min(128, M_TOK - 128 * m)
                pl = p3aps.tile([128, V0], f32, tag="pl0", name=f"pl0_{m}", bufs=2)
                for k in range(4):
                    nc.tensor.matmul(
                        out=pl[0:rows, :],
                        lhsT=h_allT[k][:, 128 * m:128 * m + rows],
                        rhs=wl0[:, V0 * k: V0 * (k + 1)],
                        start=(k == 0), stop=(k == 3))
                stg = p3ast.tile([128, V0], f32, tag="stg0", name=f"stg0_{m}")
                nc.vector.tensor_tensor(out=stg[0:rows, :], in0=pl[0:rows, :],
                                        in1=blin0[0:rows, :], op=ADD)
                nc.sync.dma_start(out=out_d[128 * m:128 * m + rows, 0:V0],
                                  in_=stg[0:rows, :])

            with tc.tile_pool(name="rps", bufs=1, space="PSUM") as rps:
                # chunk order: f(1) first, then i(0), g(2), o(3)
                for s in range(S):
                    m, a = s // 4, s % 4
                    if s == 0:
                        lhs = [hT0[:, BL * k:BL * (k + 1)] for k in range(4)]
                    else:
                        lhs = [h_allT[k][:, BL * (s - 1):BL * s] for k in range(4)]
                    pg = rps.tile([BL, G4], f32, tag="pg_rec")
                    for n in (1, 0, 2, 3):
                        for k in range(4):
                            nc.tensor.matmul(
                                out=pg[:, 512 * n:512 * (n + 1)], lhsT=lhs[k],
                                rhs=whhT[:, G4 * k + 512 * n: G4 * k + 512 * (n + 1)],
                                start=(k == 0), stop=False)
                        nc.tensor.matmul(
                            out=pg[:, 512 * n:512 * (n + 1)],
                            lhsT=sel_sb[:, BL * a:BL * (a + 1)],
                            rhs=gx_tiles[m][:, 512 * n:512 * (n + 1)],
                            start=False, stop=True)
                    # activations (i=0, f=1, g=2, o=3); i+f fused in one op
                    nc.scalar.activation(out=act_sb[:, 0:1024], in_=pg[:, 0:1024], func=SIG)
                    nc.scalar.activation(out=act_sb[:, 1024:1536], in_=pg[:, 1024:1536], func=TANH)
                    nc.scalar.activation(out=act_sb[:, 1536:2048], in_=pg[:, 1536:2048], func=SIG)
                    # cell: t2 on DVE, t1 on GPSIMD (parallel engines)
                    nc.vector.tensor_tensor(out=t2_sb[:], in0=act_sb[:, 512:1024], in1=c_sb[:], op=MUL)
                    nc.gpsimd.tensor_tensor(out=t1_sb[:], in0=act_sb[:, 0:512], in1=act_sb[:, 1024:1536], op=MUL)
                    # c, tanh(c), h in halves so transposes start earlier
                    pt4 = tps.tile([128, 128], f32r, tag="pt", bufs=2)
                    for half in range(2):
                        lo, hi = 256 * half, 256 * (half + 1)
                        nc.vector.tensor_tensor(out=c_sb[:, lo:hi], in0=t1_sb[:, lo:hi],
                                                in1=t2_sb[:, lo:hi], op=ADD)
                        nc.scalar.activation(out=th_sb[:, lo:hi], in_=c_sb[:, lo:hi], func=TANH)
                        nc.vector.tensor_tensor(out=h_sb[:, lo:hi], in0=act_sb[:, 1536 + lo:1536 + hi],
                                                in1=th_sb[:, lo:hi], op=MUL)
                        for k in (2 * half, 2 * half + 1):
                            nc.tensor.transpose(out=pt4[0:128, BL * k:BL * (k + 1)],
                                                in_=h_sb[:, 128 * k:128 * (k + 1)],
                                                identity=id128[0:BL, 0:BL])
                        # copies split DVE/ACT
                        k0, k1 = 2 * half, 2 * half + 1
                        nc.vector.tensor_copy(out=h_allT[k0][:, BL * s:BL * (s + 1)],
                                              in_=pt4[0:128, BL * k0:BL * (k0 + 1)])
                        nc.vector.tensor_copy(out=h_allT[k1][:, BL * s:BL * (s + 1)],
                                              in_=pt4[0:128, BL * k1:BL * (k1 + 1)])
                    if s % 4 == 3:
                        emit_super0(s // 4)
                    elif s == S - 1:
                        emit_super0(NMT - 1)

            tps.release()
            gxp.release()
            p3ast.release()
            p3aps.release()
            p3a.release()

            # ---------- phase 3: logits GEMM ----------
            with tc.tile_pool(name="p3", bufs=1) as p3, \
                 tc.tile_pool(name="p3w", bufs=2) as p3w, \
                 tc.tile_pool(name="p3st", bufs=3) as p3st, \
                 tc.tile_pool(name="p3ps", bufs=2, space="PSUM") as p3ps:
                blinb = p3.tile([128, V - 512], f32r, tag="blinb")
                nc.sync.dma_start(out=blinb[:], in_=blinb_d[:, 512:V])
                sup_bounds = [(512 + 1898 * i, min(512 + 1898 * (i + 1), V)) for i in range(5)]

                def load_wl(ns):
                    # issue weight loads from ACT so they don't queue behind
                    # the SP-issued output stores
                    c0, c1 = sup_bounds[ns]
                    wl = p3w.tile([128, 4 * VSUP], f32r, tag="wl", name=f"wl{ns}")
                    for k in range(4):
                        nc.scalar.dma_start(out=wl[:, VSUP * k:VSUP * k + (c1 - c0)],
                                            in_=wlinT_d[128 * k:128 * (k + 1), c0:c1])
                    return wl

                wl_next = load_wl(0)
                for ns, (c0, c1) in enumerate(sup_bounds):
                    w_sup = c1 - c0
                    chunks = []
                    off = 0
                    while off < w_sup:
                        chunks.append((off, min(512, w_sup - off)))
                        off += 512
                    wl = wl_next
                    for m in range(NMT):
                        if m == 1 and ns + 1 < len(sup_bounds):
                            wl_next = load_wl(ns + 1)
                        rows = min(128, M_TOK - 128 * m)
                        pl = p3ps.tile([128, VSUP], f32, tag="pl")
                        for off, width in chunks:
                            for k in range(4):
                                nc.tensor.matmul(
                                    out=pl[0:rows, off:off + width],
                                    lhsT=h_allT[k][:, 128 * m:128 * m + rows],
                                    rhs=wl[:, VSUP * k + off: VSUP * k + off + width],
                                    start=(k == 0), stop=(k == 3))
                        stg = p3st.tile([128, VSUP], f32, tag="stg")
                        nc.vector.tensor_tensor(out=stg[0:rows, 0:w_sup], in0=pl[0:rows, 0:w_sup],
                                                in1=blinb[0:rows, c0 - 512:c1 - 512], op=ADD)
                        nc.sync.dma_start(out=out_d[128 * m:128 * m + rows, c0:c1],
                                          in_=stg[0:rows, 0:w_sup])


    nc.compile()
    return nc


def _prep_host(caps, latent, embed, W_ih, W_hh, b_ih, b_hh, W_lin, b_lin):
    caps = np.asarray(caps).astype(np.int32)
    latent = np.asarray(latent, dtype=np.float32)
    embed = np.ascontiguousarray(np.asarray(embed, dtype=np.float32))
    wihT = np.ascontiguousarray(np.asarray(W_ih, dtype=np.float32).T)     # [E, 4H]
    whhT = np.ascontiguousarray(np.asarray(W_hh, dtype=np.float32).T)     # [H, 4H]
    bias = (np.asarray(b_ih, dtype=np.float32) + np.asarray(b_hh, dtype=np.float32))
    biasb = np.ascontiguousarray(np.broadcast_to(bias[None, :], (128, 4 * H)))
    wlinT = np.ascontiguousarray(np.asarray(W_lin, dtype=np.float32).T)   # [H, V]
    blinb = np.ascontiguousarray(np.broadcast_to(
        np.asarray(b_lin, dtype=np.float32)[None, :], (128, V)))
    sel = np.zeros((128, 4 * BL), dtype=np.float32)
    for a in range(4):
        for b in range(BL):
            sel[32 * a + b, BL * a + b] = 1.0
    id128 = np.eye(128, dtype=np.float32)

    in_maps = []
    for c in range(NCORES):
        caps_sh = caps[c * BL:(c + 1) * BL]                     # [32, 32]
        tok_flat = caps_sh[:, :S].T.reshape(M_TOK)            # t-major [992]
        tok_pad = np.zeros(NMT * 128, dtype=np.int32)
        tok_pad[:M_TOK] = tok_flat
        tok = np.ascontiguousarray(tok_pad.reshape(NMT, 128).T)  # [128, NMT]
        in_maps.append(dict(
            emb=embed, wihT=wihT, whhT=whhT, biasb=biasb, wlinT=wlinT,
            blinb=blinb, tok=tok, lat=np.ascontiguousarray(latent[c * BL:(c + 1) * BL]),
            sel=sel, id128=id128,
        ))
    return in_maps


def kernel(caps, latent, embed, W_ih, W_hh, b_ih, b_hh, W_lin, b_lin):
    from concourse.bass_utils import run_bass_kernel_spmd

    if "nc" not in _CACHE:
        _CACHE["nc"] = _build()
    nc = _CACHE["nc"]

    in_maps = _prep_host(caps, latent, embed, W_ih, W_hh, b_ih, b_hh, W_lin, b_lin)
    res = run_bass_kernel_spmd(nc, in_maps, core_ids=list(range(NCORES)))
    out = np.zeros((T, B_FULL, V), dtype=np.float32)
    for c in range(NCORES):
        shard = res.results[c]["out"].reshape(S, BL, V)
        out[1:, c * BL:(c + 1) * BL, :] = shard
    return out



# revision 32
# speedup vs baseline: 2.2254x; 2.2254x over previous
"""Teacher-forced decoder LSTM on 8 TRN2 NeuronCores.

Problem: B=256, T=32, V=10000, E=H=512 (fp32).
  step s in 0..30: x = embed[caps[:, s]]
                   gates = x@W_ih.T + h@W_hh.T + b     (i,f,g,o)
                   c = sig(f)*c + sig(i)*tanh(g); h = sig(o)*tanh(c)
                   out[s+1] = h@W_lin.T + b_lin
  out[0] = 0.  Output [T, B, V].

Sharding: data-parallel over batch, B_local=32 per core.

Layout: the recurrence runs fully TRANSPOSED (gate/hidden dims on
partitions, batch on the free axis) so each recurrent matmul moves only
32 columns. bf16 weights/activations in the gate path give 1 cyc/row on
the PE at any free size; the logits GEMM stays fp32r off the f32 copy of
h for accuracy.

  phase 1: gather X = embed[tok], PE-transpose, GxT = W_ih@X.T + b as
     bf16 tiles [128, (q)(tok)] per gate type (bias folded in).
  phase 2 (recurrent): per step 4 whole-bank gxT-inject matmuls (start
     the psum group) + 64 W_hh matmuls, all [*, 32/128]-moving bf16; ACT
     sig/tanh straight from PSUM; DVE/Pool cell update; h written
     directly into transposed history (bf16 for the recurrence, f32r for
     the logits GEMM) - no per-step transposes. Logits cols 0:1024 are
     interleaved into the PE stall windows between steps, and the first
     two phase-3 weight super-chunks prefetch on the idle SP DMA queue.
  phase 3: logits cols 1024:10000 as fp32r GEMM streamed per ~1800-col
     super-chunk, stores alternating SP/Pool DMA queues.
"""
import numpy as np

B_FULL, T, V, E, H = 256, 32, 10000, 512, 512
NCORES = 8
BL = B_FULL // NCORES          # 32 batch per core
S = T - 1                      # 31 recurrent steps
M_TOK = S * BL                 # 992 token rows per core
G4 = 4 * H                     # 2048 gate dims
NMT = (M_TOK + 127) // 128     # 8 token m-tiles (last is 96 rows)
V0 = 2048                      # vocab cols done inside phase 2
W3 = 1536                      # phase-3 super-chunk width (3 x 512)
NS3 = 6                        # phase-3 super count (5 x 1536 + 272)

_CACHE = {}


def _build():
    import concourse.bacc as bacc
    import concourse.mybir as mybir
    from concourse.tile import TileContext
    import concourse.bass as bass

    f32 = mybir.dt.float32
    f32r = mybir.dt.float32r
    bf16 = mybir.dt.bfloat16
    i32 = mybir.dt.int32
    SIG = mybir.ActivationFunctionType.Sigmoid
    TANH = mybir.ActivationFunctionType.Tanh
    ADD = mybir.AluOpType.add
    MUL = mybir.AluOpType.mult

    nc = bacc.Bacc()

    emb_d = nc.dram_tensor("emb", [V, E], f32r, kind="ExternalInput")
    wihT_d = nc.dram_tensor("wihT", [E, G4], bf16, kind="ExternalInput")
    whhT_d = nc.dram_tensor("whhT", [H, G4], bf16, kind="ExternalInput")
    biasq_d = nc.dram_tensor("biasq", [128, G4], bf16, kind="ExternalInput")
    wlinT_d = nc.dram_tensor("wlinT", [H, V], f32r, kind="ExternalInput")
    blinb_d = nc.dram_tensor("blinb", [128, V], bf16, kind="ExternalInput")
    tok_d = nc.dram_tensor("tok", [128, NMT], i32, kind="ExternalInput")
    lat_d = nc.dram_tensor("lat", [BL, H], f32r, kind="ExternalInput")
    id128_d = nc.dram_tensor("id128", [128, 128], f32r, kind="ExternalInput")
    id128b_d = nc.dram_tensor("id128b", [128, 128], bf16, kind="ExternalInput")
    out_d = nc.dram_tensor("out", [M_TOK, V], f32, kind="ExternalOutput")

    GATE_ORDER = (2, 0, 1, 3)   # g, i, f, o: start the tanh_g chain early

    with TileContext(nc) as tc:
        with tc.tile_pool(name="const", bufs=1) as cp, \
             tc.tile_pool(name="state", bufs=1) as st:

            # ---------- constants ----------
            tok_sb = cp.tile([128, NMT], i32, tag="tok_sb")
            nc.sync.dma_start(out=tok_sb[:], in_=tok_d[:])
            id128 = cp.tile([128, 128], f32r, tag="id128")
            nc.sync.dma_start(out=id128[:], in_=id128_d[:])
            lat_sb = cp.tile([BL, H], f32r, tag="lat_sb")
            nc.sync.dma_start(out=lat_sb[:], in_=lat_d[:])
            id128b = cp.tile([128, 128], bf16, tag="id128b")
            nc.sync.dma_start(out=id128b[:], in_=id128b_d[:])

            # ---------- state ----------
            hall_f = st.tile([128, 4 * M_TOK], f32r, tag="hall_f")
            hall_b = st.tile([128, 4 * M_TOK], bf16, tag="hall_b")
            hT0 = st.tile([128, 4 * BL], bf16, tag="hT0")   # col = 32k + b
            cT = st.tile([128, 4 * BL], f32, tag="cT")
            nc.vector.memset(cT[:], 0.0)
            actif = st.tile([128, 8 * BL], f32, tag="actif")
            actg = st.tile([128, 4 * BL], f32, tag="actg")
            acto = st.tile([128, 4 * BL], f32, tag="acto")
            t1_sb = st.tile([128, 4 * BL], f32, tag="t1_sb")
            t2_sb = st.tile([128, 4 * BL], f32, tag="t2_sb")
            th_sb = st.tile([128, 4 * BL], f32, tag="th_sb")
            # preload sigmoid/tanh activation tables during setup
            nc.scalar.activation(out=t1_sb[0:1, 0:1], in_=cT[0:1, 0:1], func=SIG)
            nc.scalar.activation(out=t2_sb[0:1, 0:1], in_=cT[0:1, 0:1], func=TANH)

            # phase-3 weight/bias rings (allocated first so they outlive gxp)
            p3w = tc.alloc_tile_pool(name="p3w", bufs=2)
            p3b = tc.alloc_tile_pool(name="p3b", bufs=3)
            sup_bounds = [(V0 + W3 * i, min(V0 + W3 * (i + 1), V)) for i in range(NS3)]

            def load_wl(ns, eng):
                c0, c1 = sup_bounds[ns]
                wl = p3w.tile([128, 4 * W3], f32r, tag="wl", name=f"wl{ns}")
                for k in range(4):
                    eng.dma_start(out=wl[:, W3 * k:W3 * k + (c1 - c0)],
                                  in_=wlinT_d[128 * k:128 * (k + 1), c0:c1])
                return wl

            def load_blin(ns, eng):
                c0, c1 = sup_bounds[ns]
                bl = p3b.tile([128, W3], bf16, tag="bl3", name=f"bl3_{ns}")
                eng.dma_start(out=bl[:, 0:c1 - c0], in_=blinb_d[:, c0:c1])
                return bl

            def wl_pieces(ns, wl):
                # split one wl super-load into small DMAs that drip onto the
                # SP queue between recurrence steps (big transfers would
                # block the interleaved logit stores behind them)
                c0, c1 = sup_bounds[ns]
                ps = []
                for k in range(4):
                    for h0 in range(0, c1 - c0, 512):
                        h1 = min(h0 + 512, c1 - c0)
                        ps.append((wl[:, W3 * k + h0:W3 * k + h1],
                                   wlinT_d[128 * k:128 * (k + 1), c0 + h0:c0 + h1]))
                return ps

            def blin_pieces(ns, bl):
                c0, c1 = sup_bounds[ns]
                ps = []
                for h0 in range(0, c1 - c0, 512):
                    h1 = min(h0 + 512, c1 - c0)
                    ps.append((bl[:, h0:h1], blinb_d[:, c0 + h0:c0 + h1]))
                return ps

            # long-lived recurrence inputs (released before phase 3)
            gxp = tc.alloc_tile_pool(name="gxp", bufs=1)
            whh_sb = gxp.tile([128, 4 * G4], bf16, tag="whh_sb")
            nc.sync.dma_start(out=whh_sb[:].rearrange("p (k m) -> p k m", k=4),
                              in_=whhT_d.rearrange("(k p) m -> p k m", k=4))
            gxT = [gxp.tile([128, 4 * M_TOK], bf16, tag=f"gxT{t}", name=f"gxT{t}")
                   for t in range(4)]
            # super0 (vocab cols 0:V0) weights, interleaved into phase 2
            wl0 = gxp.tile([128, 4 * V0], f32r, tag="wl0")
            blin0 = gxp.tile([128, V0], bf16, tag="blin0")
            nc.sync.dma_start(out=blin0[:], in_=blinb_d[:, 0:V0])

            spp = tc.alloc_tile_pool(name="spp", bufs=2, space="PSUM")
            sst = tc.alloc_tile_pool(name="sst", bufs=2)

            # ---------- phase 1: gather X, transpose, GxT = W_ih@X.T + b ----------
            with tc.tile_pool(name="p1", bufs=1) as p1, \
                 tc.tile_pool(name="p1ps", bufs=2, space="PSUM") as p1ps, \
                 tc.tile_pool(name="xps", bufs=2, space="PSUM") as xps, \
                 tc.tile_pool(name="tpsum", bufs=1, space="PSUM") as tps:
                # W_ih in 4 k-chunk DMAs so the first GEMM starts early;
                # wl0 after it on the same queue (not needed until step 4)
                wih_sb = p1.tile([128, 4 * G4], bf16, tag="wih_sb")
                for k in range(4):
                    nc.scalar.dma_start(out=wih_sb[:, G4 * k:G4 * (k + 1)],
                                        in_=wihT_d[128 * k:128 * (k + 1), :])
                for k in range(4):
                    nc.scalar.dma_start(out=wl0[:, V0 * k:V0 * (k + 1)],
                                        in_=wlinT_d[128 * k:128 * (k + 1), 0:V0])
                biasq = p1.tile([128, G4], bf16, tag="biasq")
                nc.sync.dma_start(out=biasq[:], in_=biasq_d[:])

                # transpose h0 = latent -> hT0 (one psum tile, one copy)
                pt0 = tps.tile([128, 128], f32r, tag="pt0")
                for k in range(4):
                    nc.tensor.transpose(out=pt0[0:128, BL * k:BL * (k + 1)],
                                        in_=lat_sb[:, 128 * k:128 * (k + 1)],
                                        identity=id128[0:BL, 0:BL])
                nc.vector.tensor_copy(out=hT0[:], in_=pt0[:])

                for m in range(NMT):
                    rows = min(128, M_TOK - 128 * m)
                    x_m = p1.tile([128, E], f32r, tag="x_m", bufs=2, name=f"x_m{m}")
                    nc.gpsimd.indirect_dma_start(
                        out=x_m[0:rows, :], out_offset=None, in_=emb_d[:],
                        in_offset=bass.IndirectOffsetOnAxis(ap=tok_sb[0:rows, m:m + 1], axis=0))
                    xp = xps.tile([128, 512], f32r, tag="xp", name=f"xp{m}")
                    for k in range(4):
                        nc.tensor.transpose(out=xp[0:128, 128 * k:128 * k + rows],
                                            in_=x_m[0:rows, 128 * k:128 * (k + 1)],
                                            identity=id128[0:rows, 0:rows])
                    xt = p1.tile([128, 512], bf16, tag="xt", bufs=2, name=f"xt{m}")
                    nc.vector.tensor_copy(
                        out=xt.rearrange("p (k c) -> p k c", k=4)[:, :, 0:rows],
                        in_=xp.rearrange("p (k c) -> p k c", k=4)[:, :, 0:rows])
                    for t in range(4):
                        pg = p1ps.tile([128, 512], f32, tag="pg1", name=f"pg1_{m}_{t}")
                        for q in range(4):
                            for k in range(4):
                                nc.tensor.matmul(
                                    out=pg[:, 128 * q:128 * q + rows],
                                    lhsT=wih_sb[:, G4 * k + 512 * t + 128 * q:
                                                G4 * k + 512 * t + 128 * (q + 1)],
                                    rhs=xt[:, 128 * k:128 * k + rows],
                                    start=(k == 0), stop=(k == 3))
                        # add bias, cast to bf16 (DVE: GPSIMD cannot read PSUM)
                        nc.vector.tensor_tensor(
                            out=gxT[t].rearrange("p (q c) -> p q c", q=4)[:, :, 128 * m:128 * m + rows],
                            in0=pg.rearrange("p (q c) -> p q c", q=4)[:, :, 0:rows],
                            in1=biasq[:, 512 * t:512 * (t + 1)].rearrange(
                                "p (q c) -> p q c", q=4)[:, :, 0:rows],
                            op=ADD)

            # ---------- phase 2: recurrence with interleaved logits cols 0:V0 ----
            # super jobs: (sup, mt, k) matmuls + fin; emitted into PE stall windows
            s0_jobs = []
            for mt in range(NMT):
                for sup in range(V0 // 512):
                    for k in range(4):
                        s0_jobs.append((sup, mt, k))
            s0_state = {"i": 0, "pl": None, "wl3": 0}

            def super0_next():
                sup, mt, k = s0_jobs[s0_state["i"]]
                s0_state["i"] += 1
                rows = min(128, M_TOK - 128 * mt)
                # deprioritized: fill PE/DVE idle slots, never delay the
                # recurrence chain (the scheduler would otherwise hoist these
                # between critical matmuls)
                with tc.high_priority(offset=-600):
                    if k == 0:
                        s0_state["pl"] = spp.tile([128, 512], f32, tag="pl0",
                                                  name=f"pl0_{sup}_{mt}")
                    pl = s0_state["pl"]
                    nc.tensor.matmul(
                        out=pl[0:rows, :],
                        lhsT=hall_f[:, M_TOK * k + 128 * mt: M_TOK * k + 128 * mt + rows],
                        rhs=wl0[:, V0 * k + 512 * sup: V0 * k + 512 * (sup + 1)],
                        start=(k == 0), stop=(k == 3))
                    if k == 3:
                        stg = sst.tile([128, 512], f32, tag="stg0", name=f"stg0_{sup}_{mt}")
                        nc.vector.tensor_tensor(out=stg[0:rows, :], in0=pl[0:rows, :],
                                                in1=blin0[0:rows, 512 * sup:512 * (sup + 1)], op=ADD)
                        nc.sync.dma_start(
                            out=out_d[128 * mt:128 * mt + rows, 512 * sup:512 * (sup + 1)],
                            in_=stg[0:rows, :])

            hall_b4 = hall_b.rearrange("p (k c) -> p k c", k=4)
            hall_f4 = hall_f.rearrange("p (k c) -> p k c", k=4)
            acto4 = acto.rearrange("p (q b) -> p q b", q=4)
            th4 = th_sb.rearrange("p (q b) -> p q b", q=4)
            # prefetch pieces for phase-3 supers 0,1 (weights + bias), SP queue
            wl3_ring = [p3w.tile([128, 4 * W3], f32r, tag="wl", name="wl0"),
                        p3w.tile([128, 4 * W3], f32r, tag="wl", name="wl1")]
            bl3_ring = [p3b.tile([128, W3], bf16, tag="bl3", name="bl3_0"),
                        p3b.tile([128, W3], bf16, tag="bl3", name="bl3_1")]
            pieces = (wl_pieces(0, wl3_ring[0]) + blin_pieces(0, bl3_ring[0])
                      + wl_pieces(1, wl3_ring[1]) + blin_pieces(1, bl3_ring[1]))
            piece_i = [0]

            with tc.tile_pool(name="rps", bufs=2, space="PSUM") as rps:
                for s in range(S):
                    # i and f share one psum bank (one group, one sig_if ACT)
                    pgif = rps.tile([128, 256], f32, tag="pgif", name=f"pgif_{s}")
                    pgg = rps.tile([128, 128], f32, tag="pgg", name=f"pgg_{s}")
                    pgo = rps.tile([128, 128], f32, tag="pgo", name=f"pgo_{s}")
                    tile_of = {0: pgif, 1: pgif, 2: pgg, 3: pgo}
                    col0 = {0: 0, 1: 128, 2: 0, 3: 0}
                    # whole-bank gxT injects: start each psum group.
                    # Independent of h -> run inside the previous step's tail.
                    for t in GATE_ORDER:
                        nc.tensor.matmul(
                            out=tile_of[t][:, col0[t]:col0[t] + 128],
                            lhsT=id128b[:],
                            rhs=gxT[t].rearrange("p (q c) -> p q c", q=4)[:, :, 32 * s:32 * (s + 1)],
                            start=(t != 1), stop=False)
                    # interleaved logits matmuls: also h(s)-independent tail filler
                    emitted = 0
                    while (emitted < 4 and s0_state["i"] < len(s0_jobs)
                           and s0_jobs[s0_state["i"]][1] < s // 4):
                        super0_next()
                        emitted += 1
                    # drip phase-3 prefetch pieces onto the SP queue
                    if s >= 4:
                        for _ in range(2):
                            if piece_i[0] < len(pieces):
                                dst, src = pieces[piece_i[0]]
                                piece_i[0] += 1
                                nc.sync.dma_start(out=dst, in_=src)
                    # W_hh @ h, order g, i, f, o; k outer so low-k matmuls can
                    # start off the first half of h
                    for t in GATE_ORDER:
                        for k in range(4):
                            for q in range(4):
                                if s == 0:
                                    rh = hT0[:, BL * k:BL * (k + 1)]
                                else:
                                    rh = hall_b[:, M_TOK * k + BL * (s - 1): M_TOK * k + BL * s]
                                nc.tensor.matmul(
                                    out=tile_of[t][:, col0[t] + 32 * q:col0[t] + 32 * (q + 1)],
                                    lhsT=whh_sb[:, G4 * k + 512 * t + 128 * q:
                                                G4 * k + 512 * t + 128 * (q + 1)],
                                    rhs=rh, start=False,
                                    stop=(q == 3 and k == 3 and t != 0))
                        if t == 2:
                            nc.scalar.activation(out=actg[:], in_=pgg[:], func=TANH)
                        elif t == 1:
                            nc.scalar.activation(out=actif[:], in_=pgif[:], func=SIG)
                    # cell update: t1 on Pool, t2 on DVE
                    nc.gpsimd.tensor_tensor(out=t1_sb[:], in0=actif[:, 0:128], in1=actg[:], op=MUL)
                    nc.vector.tensor_tensor(out=t2_sb[:], in0=actif[:, 128:256], in1=cT[:], op=MUL)
                    nc.vector.tensor_tensor(out=cT[:], in0=t1_sb[:], in1=t2_sb[:], op=ADD)
                    nc.scalar.activation(out=th_sb[:], in_=cT[:], func=TANH)
                    nc.scalar.activation(out=acto[:], in_=pgo[:], func=SIG)
                    # h (bf16 first, in halves: k0-1 unblocks the next step's
                    # low-k matmuls while k2-3 is still being written)
                    nc.vector.tensor_tensor(out=hall_b4[:, 0:2, BL * s:BL * (s + 1)],
                                            in0=acto4[:, 0:2], in1=th4[:, 0:2], op=MUL)
                    nc.vector.tensor_tensor(out=hall_b4[:, 2:4, BL * s:BL * (s + 1)],
                                            in0=acto4[:, 2:4], in1=th4[:, 2:4], op=MUL)
                    nc.vector.tensor_tensor(out=hall_f4[:, :, BL * s:BL * (s + 1)],
                                            in0=acto4[:], in1=th4[:], op=MUL)
                # drain remaining super jobs and prefetch pieces
                while s0_state["i"] < len(s0_jobs):
                    super0_next()
                while piece_i[0] < len(pieces):
                    dst, src = pieces[piece_i[0]]
                    piece_i[0] += 1
                    nc.sync.dma_start(out=dst, in_=src)

            sst.release()
            spp.release()
            gxp.release()

            # ---------- phase 3: logits cols V0:10000, fp32r ----------
            with tc.tile_pool(name="p3st", bufs=3) as p3st, \
                 tc.tile_pool(name="p3ps", bufs=2, space="PSUM") as p3ps:
                nst = 0
                for ns, (c0, c1) in enumerate(sup_bounds):
                    w_sup = c1 - c0
                    chunks = []
                    off = 0
                    while off < w_sup:
                        chunks.append((off, min(512, w_sup - off)))
                        off += 512
                    wl = wl3_ring[ns]
                    bl = bl3_ring[ns]
                    for m in range(NMT):
                        if m == 1 and ns + 2 < NS3:
                            wl3_ring.append(load_wl(ns + 2, nc.scalar))
                            bl3_ring.append(load_blin(ns + 2, nc.scalar))
                        rows = min(128, M_TOK - 128 * m)
                        pl = p3ps.tile([128, W3], f32, tag="pl")
                        for off, width in chunks:
                            for k in range(4):
                                nc.tensor.matmul(
                                    out=pl[0:rows, off:off + width],
                                    lhsT=hall_f[:, M_TOK * k + 128 * m: M_TOK * k + 128 * m + rows],
                                    rhs=wl[:, W3 * k + off: W3 * k + off + width],
                                    start=(k == 0), stop=(k == 3))
                        stg = p3st.tile([128, W3], f32, tag="stg")
                        nc.vector.tensor_tensor(out=stg[0:rows, 0:w_sup], in0=pl[0:rows, 0:w_sup],
                                                in1=bl[0:rows, 0:w_sup], op=ADD)
                        # alternate store queues: SP and Pool
                        eng = nc.sync if nst % 2 == 0 else nc.gpsimd
                        nst += 1
                        eng.dma_start(out=out_d[128 * m:128 * m + rows, c0:c1],
                                      in_=stg[0:rows, 0:w_sup])

            p3b.release()
            p3w.release()

    nc.compile()
    return nc


def _prep_host(caps, latent, embed, W_ih, W_hh, b_ih, b_hh, W_lin, b_lin):
    import ml_dtypes
    bf = ml_dtypes.bfloat16
    caps = np.asarray(caps).astype(np.int32)
    latent = np.asarray(latent, dtype=np.float32)
    embed = np.ascontiguousarray(np.asarray(embed, dtype=np.float32))
    wihT = np.ascontiguousarray(np.asarray(W_ih, dtype=np.float32).T.astype(bf))  # [E, 4H]
    whhT = np.ascontiguousarray(np.asarray(W_hh, dtype=np.float32).T.astype(bf))  # [H, 4H]
    bias = (np.asarray(b_ih, dtype=np.float32) + np.asarray(b_hh, dtype=np.float32))
    # biasq[p, c] = bias[(c//128)*128 + p]
    blk = bias.reshape(G4 // 128, 128)            # [16, 128]
    biasq = np.ascontiguousarray(
        np.broadcast_to(blk.T[:, :, None], (128, G4 // 128, 128))
        .reshape(128, G4).astype(bf))
    wlinT = np.ascontiguousarray(np.asarray(W_lin, dtype=np.float32).T)   # [H, V]
    blinb = np.ascontiguousarray(np.broadcast_to(
        np.asarray(b_lin, dtype=np.float32)[None, :], (128, V)).astype(bf))
    id128 = np.eye(128, dtype=np.float32)
    id128b = np.eye(128).astype(bf)

    in_maps = []
    for c in range(NCORES):
        caps_sh = caps[c * BL:(c + 1) * BL]                   # [32, 32]
        tok_flat = caps_sh[:, :S].T.reshape(M_TOK)            # t-major [992]
        tok_pad = np.zeros(NMT * 128, dtype=np.int32)
        tok_pad[:M_TOK] = tok_flat
        tok = np.ascontiguousarray(tok_pad.reshape(NMT, 128).T)  # [128, NMT]
        in_maps.append(dict(
            emb=embed, wihT=wihT, whhT=whhT, biasq=biasq, wlinT=wlinT,
            blinb=blinb, tok=tok, lat=np.ascontiguousarray(latent[c * BL:(c + 1) * BL]),
            id128=id128, id128b=id128b,
        ))
    return in_maps


def kernel(caps, latent, embed, W_ih, W_hh, b_ih, b_hh, W_lin, b_lin):
    from concourse.bass_utils import run_bass_kernel_spmd

    if "nc" not in _CACHE:
        _CACHE["nc"] = _build()
    nc = _CACHE["nc"]

    in_maps = _prep_host(caps, latent, embed, W_ih, W_hh, b_ih, b_hh, W_lin, b_lin)
    res = run_bass_kernel_spmd(nc, in_maps, core_ids=list(range(NCORES)))
    out = np.zeros((T, B_FULL, V), dtype=np.float32)
    for c in range(NCORES):
        shard = res.results[c]["out"].reshape(S, BL, V)
        out[1:, c * BL:(c + 1) * BL, :] = shard
    return out


# revision 52
# speedup vs baseline: 2.3802x; 1.0695x over previous
"""Teacher-forced decoder LSTM on 8 TRN2 NeuronCores.

Problem: B=256, T=32, V=10000, E=H=512 (fp32).
  step s in 0..30: x = embed[caps[:, s]]
                   gates = x@W_ih.T + h@W_hh.T + b     (i,f,g,o)
                   c = sig(f)*c + sig(i)*tanh(g); h = sig(o)*tanh(c)
                   out[s+1] = h@W_lin.T + b_lin
  out[0] = 0.  Output [T, B, V].

Sharding: data-parallel over batch, B_local=32 per core.

Layout: the recurrence runs fully TRANSPOSED (gate/hidden dims on
partitions, batch on the free axis) so each recurrent matmul moves only
32 columns. bf16 weights/activations in the gate path give 1 cyc/row on
the PE at any free size; the logits GEMM stays fp32r off the f32 copy of
h for accuracy.

  phase 1: gather X = embed[tok], PE-transpose, GxT = W_ih@X.T + b as
     bf16 tiles [128, (q)(tok)] per gate type (bias folded in).
  phase 2 (recurrent): per step 4 whole-bank gxT-inject matmuls (start
     the psum group) + 64 W_hh matmuls, all [*, 32/128]-moving bf16; ACT
     sig/tanh straight from PSUM; DVE/Pool cell update; h written
     directly into transposed history (bf16 for the recurrence, f32r for
     the logits GEMM) - no per-step transposes. Logits cols 0:1024 are
     interleaved into the PE stall windows between steps, and the first
     two phase-3 weight super-chunks prefetch on the idle SP DMA queue.
  phase 3: logits cols 1024:10000 as fp32r GEMM streamed per ~1800-col
     super-chunk, stores alternating SP/Pool DMA queues.
"""
import numpy as np

B_FULL, T, V, E, H = 256, 32, 10000, 512, 512
NCORES = 8
BL = B_FULL // NCORES          # 32 batch per core
S = T - 1                      # 31 recurrent steps
M_TOK = S * BL                 # 992 token rows per core
G4 = 4 * H                     # 2048 gate dims
NMT = (M_TOK + 127) // 128     # 8 token m-tiles (last is 96 rows)
V0 = 2048                      # vocab cols done inside phase 2
W3 = 1536                      # phase-3 super-chunk width (3 x 512)
NS3 = 6                        # phase-3 super count (5 x 1536 + 272)

_CACHE = {}


def _build():
    import concourse.bacc as bacc
    import concourse.mybir as mybir
    from concourse.tile import TileContext
    import concourse.bass as bass

    f32 = mybir.dt.float32
    f32r = mybir.dt.float32r
    bf16 = mybir.dt.bfloat16
    i32 = mybir.dt.int32
    SIG = mybir.ActivationFunctionType.Sigmoid
    TANH = mybir.ActivationFunctionType.Tanh
    ADD = mybir.AluOpType.add
    MUL = mybir.AluOpType.mult

    nc = bacc.Bacc()

    emb_d = nc.dram_tensor("emb", [V, E], f32r, kind="ExternalInput")
    wihT_d = nc.dram_tensor("wihT", [E, G4], bf16, kind="ExternalInput")
    whhT_d = nc.dram_tensor("whhT", [H, G4], bf16, kind="ExternalInput")
    biasq_d = nc.dram_tensor("biasq", [128, G4], bf16, kind="ExternalInput")
    wlinT_d = nc.dram_tensor("wlinT", [H, V], f32r, kind="ExternalInput")
    blinb_d = nc.dram_tensor("blinb", [128, V], bf16, kind="ExternalInput")
    tok_d = nc.dram_tensor("tok", [128, NMT], i32, kind="ExternalInput")
    lat_d = nc.dram_tensor("lat", [BL, H], f32r, kind="ExternalInput")
    id128_d = nc.dram_tensor("id128", [128, 128], f32r, kind="ExternalInput")
    id128b_d = nc.dram_tensor("id128b", [128, 128], bf16, kind="ExternalInput")
    out_d = nc.dram_tensor("out", [M_TOK, V], f32, kind="ExternalOutput")

    GATE_ORDER = (2, 0, 1, 3)   # g, i, f, o: start the tanh_g chain early

    with TileContext(nc) as tc:
        with tc.tile_pool(name="const", bufs=1) as cp, \
             tc.tile_pool(name="state", bufs=1) as st:

            # ---------- constants ----------
            tok_sb = cp.tile([128, NMT], i32, tag="tok_sb")
            nc.sync.dma_start(out=tok_sb[:], in_=tok_d[:])
            id128 = cp.tile([128, 128], f32r, tag="id128")
            nc.sync.dma_start(out=id128[:], in_=id128_d[:])
            lat_sb = cp.tile([BL, H], f32r, tag="lat_sb")
            nc.sync.dma_start(out=lat_sb[:], in_=lat_d[:])
            id128b = cp.tile([128, 128], bf16, tag="id128b")
            nc.sync.dma_start(out=id128b[:], in_=id128b_d[:])

            # ---------- state ----------
            hall_f = st.tile([128, 4 * M_TOK], f32r, tag="hall_f")
            hall_b = st.tile([128, 4 * M_TOK], bf16, tag="hall_b")
            hT0 = st.tile([128, 4 * BL], bf16, tag="hT0")   # col = 32k + b
            cT = st.tile([128, 4 * BL], f32, tag="cT")
            nc.vector.memset(cT[:], 0.0)
            actif = st.tile([128, 8 * BL], f32, tag="actif")
            actg = st.tile([128, 4 * BL], f32, tag="actg")
            acto = st.tile([128, 4 * BL], f32, tag="acto")
            t1_sb = st.tile([128, 4 * BL], f32, tag="t1_sb")
            t2_sb = st.tile([128, 4 * BL], f32, tag="t2_sb")
            th_sb = st.tile([128, 4 * BL], f32, tag="th_sb")
            # preload sigmoid/tanh activation tables during setup
            nc.scalar.activation(out=t1_sb[0:1, 0:1], in_=cT[0:1, 0:1], func=SIG)
            nc.scalar.activation(out=t2_sb[0:1, 0:1], in_=cT[0:1, 0:1], func=TANH)

            # phase-3 weight/bias rings (allocated first so they outlive gxp)
            p3w = tc.alloc_tile_pool(name="p3w", bufs=2)
            p3b = tc.alloc_tile_pool(name="p3b", bufs=3)
            sup_bounds = [(V0 + W3 * i, min(V0 + W3 * (i + 1), V)) for i in range(NS3)]

            def load_wl(ns, eng):
                c0, c1 = sup_bounds[ns]
                wl = p3w.tile([128, 4 * W3], f32r, tag="wl", name=f"wl{ns}")
                for k in range(4):
                    eng.dma_start(out=wl[:, W3 * k:W3 * k + (c1 - c0)],
                                  in_=wlinT_d[128 * k:128 * (k + 1), c0:c1])
                return wl

            def load_blin(ns, eng):
                c0, c1 = sup_bounds[ns]
                bl = p3b.tile([128, W3], bf16, tag="bl3", name=f"bl3_{ns}")
                eng.dma_start(out=bl[:, 0:c1 - c0], in_=blinb_d[:, c0:c1])
                return bl

            def wl_pieces(ns, wl):
                # split one wl super-load into small DMAs that drip onto the
                # SP queue between recurrence steps (big transfers would
                # block the interleaved logit stores behind them)
                c0, c1 = sup_bounds[ns]
                ps = []
                for k in range(4):
                    for h0 in range(0, c1 - c0, 512):
                        h1 = min(h0 + 512, c1 - c0)
                        ps.append((wl[:, W3 * k + h0:W3 * k + h1],
                                   wlinT_d[128 * k:128 * (k + 1), c0 + h0:c0 + h1]))
                return ps

            def blin_pieces(ns, bl):
                c0, c1 = sup_bounds[ns]
                ps = []
                for h0 in range(0, c1 - c0, 512):
                    h1 = min(h0 + 512, c1 - c0)
                    ps.append((bl[:, h0:h1], blinb_d[:, c0 + h0:c0 + h1]))
                return ps

            # super0 weights/bias live through phase 3 (mt7 jobs run there)
            wlp = tc.alloc_tile_pool(name="wlp", bufs=1)
            # long-lived recurrence inputs (released before phase 3)
            gxp = tc.alloc_tile_pool(name="gxp", bufs=1)
            whh_sb = gxp.tile([128, 4 * G4], bf16, tag="whh_sb")
            gxT = [gxp.tile([128, 4 * M_TOK], bf16, tag=f"gxT{t}", name=f"gxT{t}")
                   for t in range(4)]
            # super0 (vocab cols 0:V0) weights, interleaved into phase 2
            wl0 = wlp.tile([128, 4 * V0], f32r, tag="wl0")
            blin0 = wlp.tile([128, V0], bf16, tag="blin0")

            spp = tc.alloc_tile_pool(name="spp", bufs=2, space="PSUM")
            sst = tc.alloc_tile_pool(name="sst", bufs=4)

            # ---------- phase 1: gather X, transpose, GxT = W_ih@X.T + b ----------
            with tc.tile_pool(name="p1", bufs=1) as p1, \
                 tc.tile_pool(name="p1ps", bufs=2, space="PSUM") as p1ps, \
                 tc.tile_pool(name="xps", bufs=2, space="PSUM") as xps, \
                 tc.tile_pool(name="tpsum", bufs=1, space="PSUM") as tps:
                # W_ih in 4 k-chunk DMAs so the first GEMM starts early;
                # wl0 after it on the same queue (not needed until step 4)
                wih_sb = p1.tile([128, 4 * G4], bf16, tag="wih_sb")
                for k in range(4):
                    eng = nc.scalar if k % 2 == 0 else nc.sync
                    eng.dma_start(out=wih_sb[:, G4 * k:G4 * (k + 1)],
                                  in_=wihT_d[128 * k:128 * (k + 1), :])
                biasq = p1.tile([128, G4], bf16, tag="biasq")
                nc.sync.dma_start(out=biasq[:], in_=biasq_d[:])
                # whh/blin0/wl0 after wih+biasq: not needed until step 0+
                nc.sync.dma_start(out=whh_sb[:].rearrange("p (k m) -> p k m", k=4),
                                  in_=whhT_d.rearrange("(k p) m -> p k m", k=4))
                nc.sync.dma_start(out=blin0[:], in_=blinb_d[:, 0:V0])
                for k in range(4):
                    nc.scalar.dma_start(out=wl0[:, V0 * k:V0 * (k + 1)],
                                        in_=wlinT_d[128 * k:128 * (k + 1), 0:V0])

                # transpose h0 = latent -> hT0 (one psum tile, one copy)
                pt0 = tps.tile([128, 128], f32r, tag="pt0")
                for k in range(4):
                    nc.tensor.transpose(out=pt0[0:128, BL * k:BL * (k + 1)],
                                        in_=lat_sb[:, 128 * k:128 * (k + 1)],
                                        identity=id128[0:BL, 0:BL])
                nc.vector.tensor_copy(out=hT0[:], in_=pt0[:])

                for m in range(NMT):
                    rows = min(128, M_TOK - 128 * m)
                    x_m = p1.tile([128, E], f32r, tag="x_m", bufs=2, name=f"x_m{m}")
                    nc.gpsimd.indirect_dma_start(
                        out=x_m[0:rows, :], out_offset=None, in_=emb_d[:],
                        in_offset=bass.IndirectOffsetOnAxis(ap=tok_sb[0:rows, m:m + 1], axis=0))
                    xp = xps.tile([128, 512], f32r, tag="xp", name=f"xp{m}")
                    for k in range(4):
                        nc.tensor.transpose(out=xp[0:128, 128 * k:128 * k + rows],
                                            in_=x_m[0:rows, 128 * k:128 * (k + 1)],
                                            identity=id128[0:rows, 0:rows])
                    xt = p1.tile([128, 512], bf16, tag="xt", bufs=2, name=f"xt{m}")
                    nc.vector.tensor_copy(
                        out=xt.rearrange("p (k c) -> p k c", k=4)[:, :, 0:rows],
                        in_=xp.rearrange("p (k c) -> p k c", k=4)[:, :, 0:rows])
                    for t in range(4):
                        pg = p1ps.tile([128, 512], f32, tag="pg1", name=f"pg1_{m}_{t}")
                        for q in range(4):
                            for k in range(4):
                                nc.tensor.matmul(
                                    out=pg[:, 128 * q:128 * q + rows],
                                    lhsT=wih_sb[:, G4 * k + 512 * t + 128 * q:
                                                G4 * k + 512 * t + 128 * (q + 1)],
                                    rhs=xt[:, 128 * k:128 * k + rows],
                                    start=(k == 0), stop=(k == 3))
                        # add bias, cast to bf16 (DVE: GPSIMD cannot read PSUM)
                        nc.vector.tensor_tensor(
                            out=gxT[t].rearrange("p (q c) -> p q c", q=4)[:, :, 128 * m:128 * m + rows],
                            in0=pg.rearrange("p (q c) -> p q c", q=4)[:, :, 0:rows],
                            in1=biasq[:, 512 * t:512 * (t + 1)].rearrange(
                                "p (q c) -> p q c", q=4)[:, :, 0:rows],
                            op=ADD)

            # ---------- phase 2: recurrence with interleaved logits cols 0:V0 ----
            # super jobs: (sup, mt, k) matmuls + fin; emitted into PE stall windows
            s0_jobs = []
            for mt in range(NMT - 1):
                for sup in range(V0 // 512):
                    for k in range(4):
                        s0_jobs.append((sup, mt, k))
            s0_state = {"i": 0, "pl": None, "wl3": 0}

            def super0_next():
                sup, mt, k = s0_jobs[s0_state["i"]]
                s0_state["i"] += 1
                rows = min(128, M_TOK - 128 * mt)
                # deprioritized: fill PE/DVE idle slots, never delay the
                # recurrence chain (the scheduler would otherwise hoist these
                # between critical matmuls)
                with tc.high_priority(offset=-600):
                    if k == 0:
                        s0_state["pl"] = spp.tile([128, 512], f32, tag="pl0",
                                                  name=f"pl0_{sup}_{mt}")
                    pl = s0_state["pl"]
                    nc.tensor.matmul(
                        out=pl[0:rows, :],
                        lhsT=hall_f[:, M_TOK * k + 128 * mt: M_TOK * k + 128 * mt + rows],
                        rhs=wl0[:, V0 * k + 512 * sup: V0 * k + 512 * (sup + 1)],
                        start=(k == 0), stop=(k == 3))
                    if k == 3:
                        stg = sst.tile([128, 512], f32, tag="stg0", name=f"stg0_{sup}_{mt}")
                        # half-width adds: bounds how long a fin can block
                        # the critical h-write on the in-order DVE queue
                        for hh in (0, 256):
                            nc.vector.tensor_tensor(
                                out=stg[0:rows, hh:hh + 256], in0=pl[0:rows, hh:hh + 256],
                                in1=blin0[0:rows, 512 * sup + hh:512 * sup + hh + 256], op=ADD)
                        nc.sync.dma_start(
                            out=out_d[128 * mt:128 * mt + rows, 512 * sup:512 * (sup + 1)],
                            in_=stg[0:rows, :])

            hall_b4 = hall_b.rearrange("p (k c) -> p k c", k=4)
            hall_f4 = hall_f.rearrange("p (k c) -> p k c", k=4)
            acto4 = acto.rearrange("p (q b) -> p q b", q=4)
            th4 = th_sb.rearrange("p (q b) -> p q b", q=4)
            # prefetch pieces for phase-3 supers 0,1 (weights + bias), SP queue
            wl3_ring = [p3w.tile([128, 4 * W3], f32r, tag="wl", name="wl0"),
                        p3w.tile([128, 4 * W3], f32r, tag="wl", name="wl1")]
            bl3_ring = [p3b.tile([128, W3], bf16, tag="bl3", name="bl3_0"),
                        p3b.tile([128, W3], bf16, tag="bl3", name="bl3_1")]
            pieces = (wl_pieces(0, wl3_ring[0]) + blin_pieces(0, bl3_ring[0])
                      + wl_pieces(1, wl3_ring[1]) + blin_pieces(1, bl3_ring[1]))
            piece_i = [0]

            with tc.tile_pool(name="rps", bufs=2, space="PSUM") as rps:
                for s in range(S):
                    # i and f share one psum bank (one group, one sig_if ACT)
                    pgif = rps.tile([128, 256], f32, tag="pgif", name=f"pgif_{s}")
                    pgg = rps.tile([128, 128], f32, tag="pgg", name=f"pgg_{s}")
                    pgo = rps.tile([128, 128], f32, tag="pgo", name=f"pgo_{s}")
                    tile_of = {0: pgif, 1: pgif, 2: pgg, 3: pgo}
                    col0 = {0: 0, 1: 128, 2: 0, 3: 0}
                    # whole-bank gxT injects: start each psum group.
                    # Independent of h -> run inside the previous step's tail.
                    for t in GATE_ORDER:
                        nc.tensor.matmul(
                            out=tile_of[t][:, col0[t]:col0[t] + 128],
                            lhsT=id128b[:],
                            rhs=gxT[t].rearrange("p (q c) -> p q c", q=4)[:, :, 32 * s:32 * (s + 1)],
                            start=(t != 1), stop=False)
                    # interleaved logits matmuls: also h(s)-independent tail filler
                    emitted = 0
                    while (emitted < 4 and s0_state["i"] < len(s0_jobs)
                           and s0_jobs[s0_state["i"]][1] < s // 4):
                        super0_next()
                        emitted += 1
                    # drip phase-3 prefetch pieces onto the SP queue
                    if s >= 4:
                        for _ in range(2):
                            if piece_i[0] < len(pieces):
                                dst, src = pieces[piece_i[0]]
                                piece_i[0] += 1
                                nc.sync.dma_start(out=dst, in_=src)
                    # W_hh @ h, order g, i, f, o; k outer so low-k matmuls can
                    # start off the first half of h
                    for t in GATE_ORDER:
                        for k in range(4):
                            for q in range(4):
                                if s == 0:
                                    rh = hT0[:, BL * k:BL * (k + 1)]
                                else:
                                    rh = hall_b[:, M_TOK * k + BL * (s - 1): M_TOK * k + BL * s]
                                nc.tensor.matmul(
                                    out=tile_of[t][:, col0[t] + 32 * q:col0[t] + 32 * (q + 1)],
                                    lhsT=whh_sb[:, G4 * k + 512 * t + 128 * q:
                                                G4 * k + 512 * t + 128 * (q + 1)],
                                    rhs=rh, start=False,
                                    stop=(q == 3 and k == 3 and t != 0))
                        if t == 2:
                            nc.scalar.activation(out=actg[:], in_=pgg[:], func=TANH)
                        elif t == 1:
                            nc.scalar.activation(out=actif[:], in_=pgif[:], func=SIG)
                    # cell update: t1 on Pool, t2 on DVE
                    nc.gpsimd.tensor_tensor(out=t1_sb[:], in0=actif[:, 0:128], in1=actg[:], op=MUL)
                    nc.vector.tensor_tensor(out=t2_sb[:], in0=actif[:, 128:256], in1=cT[:], op=MUL)
                    nc.vector.tensor_tensor(out=cT[:], in0=t1_sb[:], in1=t2_sb[:], op=ADD)
                    # th in halves: th_lo unblocks h_lo (k0-1) a hop earlier
                    nc.scalar.activation(out=th_sb[:, 0:64], in_=cT[:, 0:64], func=TANH)
                    nc.scalar.activation(out=th_sb[:, 64:128], in_=cT[:, 64:128], func=TANH)
                    nc.scalar.activation(out=acto[:], in_=pgo[:], func=SIG)
                    # h (bf16 first, in halves: k0-1 unblocks the next step's
                    # low-k matmuls while k2-3 is still being written)
                    nc.vector.tensor_tensor(out=hall_b4[:, 0:2, BL * s:BL * (s + 1)],
                                            in0=acto4[:, 0:2], in1=th4[:, 0:2], op=MUL)
                    nc.vector.tensor_tensor(out=hall_b4[:, 2:4, BL * s:BL * (s + 1)],
                                            in0=acto4[:, 2:4], in1=th4[:, 2:4], op=MUL)
                    nc.gpsimd.tensor_tensor(out=hall_f4[:, :, BL * s:BL * (s + 1)],
                                            in0=acto4[:], in1=th4[:], op=MUL)
                # drain remaining super jobs and prefetch pieces
                while s0_state["i"] < len(s0_jobs):
                    super0_next()
                while piece_i[0] < len(pieces):
                    dst, src = pieces[piece_i[0]]
                    piece_i[0] += 1
                    nc.sync.dma_start(out=dst, in_=src)

            sst.release()
            spp.release()
            gxp.release()

            # ---------- phase 3: logits cols V0:10000, fp32r ----------
            with tc.tile_pool(name="p3st", bufs=3) as p3st, \
                 tc.tile_pool(name="p3ps", bufs=2, space="PSUM") as p3ps:
                nst = 0
                # mt7's cols 0:V0 (its h finishes only at step 30) overlap the
                # first wl stream here instead of serializing after the loop
                mt, rows = NMT - 1, M_TOK - 128 * (NMT - 1)
                for sup in range(V0 // 512):
                    pl = p3ps.tile([128, W3], f32, tag="pl", name=f"pl7_{sup}")
                    for k in range(4):
                        nc.tensor.matmul(
                            out=pl[0:rows, 0:512],
                            lhsT=hall_f[:, M_TOK * k + 128 * mt: M_TOK * k + 128 * mt + rows],
                            rhs=wl0[:, V0 * k + 512 * sup: V0 * k + 512 * (sup + 1)],
                            start=(k == 0), stop=(k == 3))
                    stg = p3st.tile([128, W3], f32, tag="stg", name=f"stg7_{sup}")
                    nc.vector.tensor_tensor(out=stg[0:rows, 0:512], in0=pl[0:rows, 0:512],
                                            in1=blin0[0:rows, 512 * sup:512 * (sup + 1)], op=ADD)
                    eng = nc.sync if nst % 2 == 0 else nc.gpsimd
                    nst += 1
                    eng.dma_start(out=out_d[128 * mt:128 * mt + rows, 512 * sup:512 * (sup + 1)],
                                  in_=stg[0:rows, 0:512])
                for ns, (c0, c1) in enumerate(sup_bounds):
                    w_sup = c1 - c0
                    chunks = []
                    off = 0
                    while off < w_sup:
                        chunks.append((off, min(512, w_sup - off)))
                        off += 512
                    wl = wl3_ring[ns]
                    bl = bl3_ring[ns]
                    for m in range(NMT):
                        if m == 1 and ns + 2 < NS3:
                            wl3_ring.append(load_wl(ns + 2, nc.scalar))
                            bl3_ring.append(load_blin(ns + 2, nc.scalar))
                        rows = min(128, M_TOK - 128 * m)
                        pl = p3ps.tile([128, W3], f32, tag="pl")
                        for off, width in chunks:
                            for k in range(4):
                                nc.tensor.matmul(
                                    out=pl[0:rows, off:off + width],
                                    lhsT=hall_f[:, M_TOK * k + 128 * m: M_TOK * k + 128 * m + rows],
                                    rhs=wl[:, W3 * k + off: W3 * k + off + width],
                                    start=(k == 0), stop=(k == 3))
                        stg = p3st.tile([128, W3], f32, tag="stg")
                        nc.vector.tensor_tensor(out=stg[0:rows, 0:w_sup], in0=pl[0:rows, 0:w_sup],
                                                in1=bl[0:rows, 0:w_sup], op=ADD)
                        # alternate store queues: SP and Pool
                        eng = nc.sync if nst % 2 == 0 else nc.gpsimd
                        nst += 1
                        eng.dma_start(out=out_d[128 * m:128 * m + rows, c0:c1],
                                      in_=stg[0:rows, 0:w_sup])

            wlp.release()
            p3b.release()
            p3w.release()

    nc.compile()
    return nc


def _prep_host(caps, latent, embed, W_ih, W_hh, b_ih, b_hh, W_lin, b_lin):
    import ml_dtypes
    bf = ml_dtypes.bfloat16
    caps = np.asarray(caps).astype(np.int32)
    latent = np.asarray(latent, dtype=np.float32)
    embed = np.ascontiguousarray(np.asarray(embed, dtype=np.float32))
    wihT = np.ascontiguousarray(np.asarray(W_ih, dtype=np.float32).T.astype(bf))  # [E, 4H]
    whhT = np.ascontiguousarray(np.asarray(W_hh, dtype=np.float32).T.astype(bf))  # [H, 4H]
    bias = (np.asarray(b_ih, dtype=np.float32) + np.asarray(b_hh, dtype=np.float32))
    # biasq[p, c] = bias[(c//128)*128 + p]
    blk = bias.reshape(G4 // 128, 128)            # [16, 128]
    biasq = np.ascontiguousarray(
        np.broadcast_to(blk.T[:, :, None], (128, G4 // 128, 128))
        .reshape(128, G4).astype(bf))
    wlinT = np.ascontiguousarray(np.asarray(W_lin, dtype=np.float32).T)   # [H, V]
    blinb = np.ascontiguousarray(np.broadcast_to(
        np.asarray(b_lin, dtype=np.float32)[None, :], (128, V)).astype(bf))
    id128 = np.eye(128, dtype=np.float32)
    id128b = np.eye(128).astype(bf)

    in_maps = []
    for c in range(NCORES):
        caps_sh = caps[c * BL:(c + 1) * BL]                   # [32, 32]
        tok_flat = caps_sh[:, :S].T.reshape(M_TOK)            # t-major [992]
        tok_pad = np.zeros(NMT * 128, dtype=np.int32)
        tok_pad[:M_TOK] = tok_flat
        tok = np.ascontiguousarray(tok_pad.reshape(NMT, 128).T)  # [128, NMT]
        in_maps.append(dict(
            emb=embed, wihT=wihT, whhT=whhT, biasq=biasq, wlinT=wlinT,
            blinb=blinb, tok=tok, lat=np.ascontiguousarray(latent[c * BL:(c + 1) * BL]),
            id128=id128, id128b=id128b,
        ))
    return in_maps


def kernel(caps, latent, embed, W_ih, W_hh, b_ih, b_hh, W_lin, b_lin):
    from concourse.bass_utils import run_bass_kernel_spmd

    if "nc" not in _CACHE:
        _CACHE["nc"] = _build()
    nc = _CACHE["nc"]

    in_maps = _prep_host(caps, latent, embed, W_ih, W_hh, b_ih, b_hh, W_lin, b_lin)
    res = run_bass_kernel_spmd(nc, in_maps, core_ids=list(range(NCORES)))
    out = np.zeros((T, B_FULL, V), dtype=np.float32)
    for c in range(NCORES):
        shard = res.results[c]["out"].reshape(S, BL, V)
        out[1:, c * BL:(c + 1) * BL, :] = shard
    return out


# revision 67
# speedup vs baseline: 2.4620x; 1.0344x over previous
"""Teacher-forced decoder LSTM on 8 TRN2 NeuronCores.

Problem: B=256, T=32, V=10000, E=H=512 (fp32).
  step s in 0..30: x = embed[caps[:, s]]
                   gates = x@W_ih.T + h@W_hh.T + b     (i,f,g,o)
                   c = sig(f)*c + sig(i)*tanh(g); h = sig(o)*tanh(c)
                   out[s+1] = h@W_lin.T + b_lin
  out[0] = 0.  Output [T, B, V].

Sharding: data-parallel over batch, B_local=32 per core.

Layout: the recurrence runs fully TRANSPOSED (gate/hidden dims on
partitions, batch on the free axis) so each recurrent matmul moves only
32 columns. bf16 weights/activations in the gate path give 1 cyc/row on
the PE at any free size; the logits GEMM stays fp32r off the f32 copy of
h for accuracy.

  phase 1: gather X = embed[tok], PE-transpose, GxT = W_ih@X.T + b as
     bf16 tiles [128, (q)(tok)] per gate type (bias folded in).
  phase 2 (recurrent): per step 4 whole-bank gxT-inject matmuls (start
     the psum group) + 64 W_hh matmuls, all [*, 32/128]-moving bf16; ACT
     sig/tanh straight from PSUM; DVE/Pool cell update; h written
     directly into transposed history (bf16 for the recurrence, f32r for
     the logits GEMM) - no per-step transposes. Logits cols 0:1024 are
     interleaved into the PE stall windows between steps, and the first
     two phase-3 weight super-chunks prefetch on the idle SP DMA queue.
  phase 3: logits cols 1024:10000 as fp32r GEMM streamed per ~1800-col
     super-chunk, stores alternating SP/Pool DMA queues.
"""
import numpy as np

B_FULL, T, V, E, H = 256, 32, 10000, 512, 512
NCORES = 8
BL = B_FULL // NCORES          # 32 batch per core
S = T - 1                      # 31 recurrent steps
M_TOK = S * BL                 # 992 token rows per core
G4 = 4 * H                     # 2048 gate dims
NMT = (M_TOK + 127) // 128     # 8 token m-tiles (last is 96 rows)
V0 = 2048                      # vocab cols done inside phase 2
W3 = 1536                      # phase-3 super-chunk width (3 x 512)
NS3 = 6                        # phase-3 super count (5 x 1536 + 272)

_CACHE = {}


def _build():
    import concourse.bacc as bacc
    import concourse.mybir as mybir
    from concourse.tile import TileContext
    import concourse.bass as bass

    f32 = mybir.dt.float32
    f32r = mybir.dt.float32r
    bf16 = mybir.dt.bfloat16
    i32 = mybir.dt.int32
    SIG = mybir.ActivationFunctionType.Sigmoid
    TANH = mybir.ActivationFunctionType.Tanh
    ADD = mybir.AluOpType.add
    MUL = mybir.AluOpType.mult

    nc = bacc.Bacc()

    emb_d = nc.dram_tensor("emb", [V, E], f32r, kind="ExternalInput")
    wihT_d = nc.dram_tensor("wihT", [E, G4], bf16, kind="ExternalInput")
    whhT_d = nc.dram_tensor("whhT", [H, G4], bf16, kind="ExternalInput")
    biasq_d = nc.dram_tensor("biasq", [128, G4], bf16, kind="ExternalInput")
    wlinT_d = nc.dram_tensor("wlinT", [H, V], f32r, kind="ExternalInput")
    blinb_d = nc.dram_tensor("blinb", [128, V], bf16, kind="ExternalInput")
    tok_d = nc.dram_tensor("tok", [128, NMT], i32, kind="ExternalInput")
    lat_d = nc.dram_tensor("lat", [BL, H], f32r, kind="ExternalInput")
    id128_d = nc.dram_tensor("id128", [128, 128], f32r, kind="ExternalInput")
    id128b_d = nc.dram_tensor("id128b", [128, 128], bf16, kind="ExternalInput")
    out_d = nc.dram_tensor("out", [M_TOK, V], f32, kind="ExternalOutput")

    GATE_ORDER = (2, 0, 1, 3)   # g, i, f, o: start the tanh_g chain early

    with TileContext(nc) as tc:
        with tc.tile_pool(name="const", bufs=1) as cp, \
             tc.tile_pool(name="state", bufs=1) as st:

            # ---------- constants ----------
            tok_sb = cp.tile([128, NMT], i32, tag="tok_sb")
            nc.sync.dma_start(out=tok_sb[:], in_=tok_d[:])
            id128 = cp.tile([128, 128], f32r, tag="id128")
            nc.sync.dma_start(out=id128[:], in_=id128_d[:])
            lat_sb = cp.tile([BL, H], f32r, tag="lat_sb")
            nc.sync.dma_start(out=lat_sb[:], in_=lat_d[:])
            id128b = cp.tile([128, 128], bf16, tag="id128b")
            nc.sync.dma_start(out=id128b[:], in_=id128b_d[:])

            # ---------- state ----------
            hall_f = st.tile([128, 4 * M_TOK], f32r, tag="hall_f")
            hall_b = st.tile([128, 4 * M_TOK], bf16, tag="hall_b")
            hT0 = st.tile([128, 4 * BL], bf16, tag="hT0")   # col = 32k + b
            cT = st.tile([128, 4 * BL], f32, tag="cT")
            nc.vector.memset(cT[:], 0.0)
            actif = st.tile([128, 8 * BL], f32, tag="actif")
            actg = st.tile([128, 4 * BL], f32, tag="actg")
            acto = st.tile([128, 4 * BL], f32, tag="acto")
            t1_sb = st.tile([128, 4 * BL], f32, tag="t1_sb")
            t2_sb = st.tile([128, 4 * BL], f32, tag="t2_sb")
            th_sb = st.tile([128, 4 * BL], f32, tag="th_sb")
            # preload sigmoid/tanh activation tables during setup
            nc.scalar.activation(out=t1_sb[0:1, 0:1], in_=cT[0:1, 0:1], func=SIG)
            nc.scalar.activation(out=t2_sb[0:1, 0:1], in_=cT[0:1, 0:1], func=TANH)

            # phase-3 weight/bias rings (allocated first so they outlive gxp)
            p3w = tc.alloc_tile_pool(name="p3w", bufs=2)
            p3b = tc.alloc_tile_pool(name="p3b", bufs=3)
            sup_bounds = [(V0 + W3 * i, min(V0 + W3 * (i + 1), V)) for i in range(NS3)]

            def load_wl(ns, eng):
                c0, c1 = sup_bounds[ns]
                wl = p3w.tile([128, 4 * W3], f32r, tag="wl", name=f"wl{ns}")
                for k in range(4):
                    eng.dma_start(out=wl[:, W3 * k:W3 * k + (c1 - c0)],
                                  in_=wlinT_d[128 * k:128 * (k + 1), c0:c1])
                return wl

            def load_blin(ns, eng):
                c0, c1 = sup_bounds[ns]
                bl = p3b.tile([128, W3], bf16, tag="bl3", name=f"bl3_{ns}")
                eng.dma_start(out=bl[:, 0:c1 - c0], in_=blinb_d[:, c0:c1])
                return bl

            def wl_pieces(ns, wl):
                # split one wl super-load into small DMAs that drip onto the
                # SP queue between recurrence steps (big transfers would
                # block the interleaved logit stores behind them)
                c0, c1 = sup_bounds[ns]
                ps = []
                for k in range(4):
                    for h0 in range(0, c1 - c0, 512):
                        h1 = min(h0 + 512, c1 - c0)
                        ps.append((wl[:, W3 * k + h0:W3 * k + h1],
                                   wlinT_d[128 * k:128 * (k + 1), c0 + h0:c0 + h1]))
                return ps

            def blin_pieces(ns, bl):
                c0, c1 = sup_bounds[ns]
                ps = []
                for h0 in range(0, c1 - c0, 512):
                    h1 = min(h0 + 512, c1 - c0)
                    ps.append((bl[:, h0:h1], blinb_d[:, c0 + h0:c0 + h1]))
                return ps

            # super0 weights/bias live through phase 3 (mt7 jobs run there)
            wlp = tc.alloc_tile_pool(name="wlp", bufs=1)
            # long-lived recurrence inputs (released before phase 3)
            gxp = tc.alloc_tile_pool(name="gxp", bufs=1)
            whh_sb = gxp.tile([128, 4 * G4], bf16, tag="whh_sb")
            gxT = [gxp.tile([128, 4 * M_TOK], bf16, tag=f"gxT{t}", name=f"gxT{t}")
                   for t in range(4)]
            # super0 (vocab cols 0:V0) weights, interleaved into phase 2
            wl0 = wlp.tile([128, 4 * V0], f32r, tag="wl0")
            blin0 = wlp.tile([128, V0], bf16, tag="blin0")
            # tensors used by deferred phase-1 m-tiles (6,7), whose GEMMs run
            # inside the empty tails of recurrence steps 0..3
            defp = tc.alloc_tile_pool(name="defp", bufs=1)
            wih_sb = defp.tile([128, 4 * G4], bf16, tag="wih_sb")
            biasq = defp.tile([128, G4], bf16, tag="biasq")
            xt_def = [defp.tile([128, 512], bf16, tag=f"xtd{m}", name=f"xtd{m}")
                      for m in (6, 7)]

            spp = tc.alloc_tile_pool(name="spp", bufs=2, space="PSUM")
            sst = tc.alloc_tile_pool(name="sst", bufs=4)

            # ---------- phase 1: gather X, transpose, GxT = W_ih@X.T + b ----------
            with tc.tile_pool(name="p1", bufs=1) as p1, \
                 tc.tile_pool(name="p1ps", bufs=2, space="PSUM") as p1ps, \
                 tc.tile_pool(name="xps", bufs=2, space="PSUM") as xps, \
                 tc.tile_pool(name="tpsum", bufs=1, space="PSUM") as tps:
                # W_ih in 4 k-chunk DMAs so the first GEMM starts early;
                # wl0 after it on the same queue (not needed until step 4)
                for k in range(4):
                    eng = nc.scalar if k % 2 == 0 else nc.sync
                    eng.dma_start(out=wih_sb[:, G4 * k:G4 * (k + 1)],
                                  in_=wihT_d[128 * k:128 * (k + 1), :])
                nc.sync.dma_start(out=biasq[:], in_=biasq_d[:])
                # whh/blin0/wl0 after wih+biasq: not needed until step 0+
                nc.sync.dma_start(out=whh_sb[:].rearrange("p (k m) -> p k m", k=4),
                                  in_=whhT_d.rearrange("(k p) m -> p k m", k=4))
                nc.sync.dma_start(out=blin0[:], in_=blinb_d[:, 0:V0])
                for k in range(4):
                    nc.scalar.dma_start(out=wl0[:, V0 * k:V0 * (k + 1)],
                                        in_=wlinT_d[128 * k:128 * (k + 1), 0:V0])

                # transpose h0 = latent -> hT0 (one psum tile, one copy)
                pt0 = tps.tile([128, 128], f32r, tag="pt0")
                for k in range(4):
                    nc.tensor.transpose(out=pt0[0:128, BL * k:BL * (k + 1)],
                                        in_=lat_sb[:, 128 * k:128 * (k + 1)],
                                        identity=id128[0:BL, 0:BL])
                nc.vector.tensor_copy(out=hT0[:], in_=pt0[:])

                def gx_gemm(m, xt, pool, tag):
                    rows = min(128, M_TOK - 128 * m)
                    for t in range(4):
                        pg = pool.tile([128, 512], f32, tag=tag, name=f"pg1_{m}_{t}")
                        for q in range(4):
                            for k in range(4):
                                nc.tensor.matmul(
                                    out=pg[:, 128 * q:128 * q + rows],
                                    lhsT=wih_sb[:, G4 * k + 512 * t + 128 * q:
                                                G4 * k + 512 * t + 128 * (q + 1)],
                                    rhs=xt[:, 128 * k:128 * k + rows],
                                    start=(k == 0), stop=(k == 3))
                        # add bias, cast to bf16 (DVE: GPSIMD cannot read PSUM)
                        nc.vector.tensor_tensor(
                            out=gxT[t].rearrange("p (q c) -> p q c", q=4)[:, :, 128 * m:128 * m + rows],
                            in0=pg.rearrange("p (q c) -> p q c", q=4)[:, :, 0:rows],
                            in1=biasq[:, 512 * t:512 * (t + 1)].rearrange(
                                "p (q c) -> p q c", q=4)[:, :, 0:rows],
                            op=ADD)

                for m in range(NMT):
                    rows = min(128, M_TOK - 128 * m)
                    x_m = p1.tile([128, E], f32r, tag="x_m", bufs=2, name=f"x_m{m}")
                    nc.gpsimd.indirect_dma_start(
                        out=x_m[0:rows, :], out_offset=None, in_=emb_d[:],
                        in_offset=bass.IndirectOffsetOnAxis(ap=tok_sb[0:rows, m:m + 1], axis=0))
                    xp = xps.tile([128, 512], f32r, tag="xp", name=f"xp{m}")
                    for k in range(4):
                        nc.tensor.transpose(out=xp[0:128, 128 * k:128 * k + rows],
                                            in_=x_m[0:rows, 128 * k:128 * (k + 1)],
                                            identity=id128[0:rows, 0:rows])
                    xt = (p1.tile([128, 512], bf16, tag="xt", bufs=2, name=f"xt{m}")
                          if m < 6 else xt_def[m - 6])
                    nc.vector.tensor_copy(
                        out=xt.rearrange("p (k c) -> p k c", k=4)[:, :, 0:rows],
                        in_=xp.rearrange("p (k c) -> p k c", k=4)[:, :, 0:rows])
                    # m 6,7: GEMM deferred into the tails of steps 0..3
                    if m < 6:
                        gx_gemm(m, xt, p1ps, "pg1")

            # deferred m 6,7 GEMMs: deprioritized fillers for steps 0..3
            # (their gxT columns are not consumed until step 24)
            with tc.high_priority(offset=-600):
                gx_gemm(6, xt_def[0], spp, "pl0")
                gx_gemm(7, xt_def[1], spp, "pl0")

            # ---------- phase 2: recurrence with interleaved logits cols 0:V0 ----
            # super jobs: (sup, mt, k) matmuls + fin; emitted into PE stall windows
            s0_jobs = []
            for mt in range(NMT - 1):
                for sup in range(V0 // 512):
                    for k in range(4):
                        s0_jobs.append((sup, mt, k))
            s0_state = {"i": 0, "pl": None, "wl3": 0}

            def super0_next():
                sup, mt, k = s0_jobs[s0_state["i"]]
                s0_state["i"] += 1
                rows = min(128, M_TOK - 128 * mt)
                # deprioritized: fill PE/DVE idle slots, never delay the
                # recurrence chain (the scheduler would otherwise hoist these
                # between critical matmuls)
                with tc.high_priority(offset=-600):
                    if k == 0:
                        s0_state["pl"] = spp.tile([128, 512], f32, tag="pl0",
                                                  name=f"pl0_{sup}_{mt}")
                    pl = s0_state["pl"]
                    nc.tensor.matmul(
                        out=pl[0:rows, :],
                        lhsT=hall_f[:, M_TOK * k + 128 * mt: M_TOK * k + 128 * mt + rows],
                        rhs=wl0[:, V0 * k + 512 * sup: V0 * k + 512 * (sup + 1)],
                        start=(k == 0), stop=(k == 3))
                    if k == 3:
                        stg = sst.tile([128, 512], f32, tag="stg0", name=f"stg0_{sup}_{mt}")
                        # half-width adds: bounds how long a fin can block
                        # the critical h-write on the in-order DVE queue
                        for hh in (0, 256):
                            nc.vector.tensor_tensor(
                                out=stg[0:rows, hh:hh + 256], in0=pl[0:rows, hh:hh + 256],
                                in1=blin0[0:rows, 512 * sup + hh:512 * sup + hh + 256], op=ADD)
                        nc.sync.dma_start(
                            out=out_d[128 * mt:128 * mt + rows, 512 * sup:512 * (sup + 1)],
                            in_=stg[0:rows, :])

            hall_b4 = hall_b.rearrange("p (k c) -> p k c", k=4)
            hall_f4 = hall_f.rearrange("p (k c) -> p k c", k=4)
            acto4 = acto.rearrange("p (q b) -> p q b", q=4)
            th4 = th_sb.rearrange("p (q b) -> p q b", q=4)
            # prefetch pieces for phase-3 supers 0,1 (weights + bias), SP queue
            wl3_ring = [p3w.tile([128, 4 * W3], f32r, tag="wl", name="wl0"),
                        p3w.tile([128, 4 * W3], f32r, tag="wl", name="wl1")]
            bl3_ring = [p3b.tile([128, W3], bf16, tag="bl3", name="bl3_0"),
                        p3b.tile([128, W3], bf16, tag="bl3", name="bl3_1")]
            pieces = (wl_pieces(0, wl3_ring[0]) + blin_pieces(0, bl3_ring[0])
                      + wl_pieces(1, wl3_ring[1]) + blin_pieces(1, bl3_ring[1]))
            piece_i = [0]

            with tc.tile_pool(name="rps", bufs=2, space="PSUM") as rps:
                for s in range(S):
                    # i and f share one psum bank (one group, one sig_if ACT)
                    pgif = rps.tile([128, 256], f32, tag="pgif", name=f"pgif_{s}")
                    pgg = rps.tile([128, 128], f32, tag="pgg", name=f"pgg_{s}")
                    pgo = rps.tile([128, 128], f32, tag="pgo", name=f"pgo_{s}")
                    tile_of = {0: pgif, 1: pgif, 2: pgg, 3: pgo}
                    col0 = {0: 0, 1: 128, 2: 0, 3: 0}
                    # whole-bank gxT injects: start each psum group.
                    # Independent of h -> run inside the previous step's tail.
                    for t in GATE_ORDER:
                        nc.tensor.matmul(
                            out=tile_of[t][:, col0[t]:col0[t] + 128],
                            lhsT=id128b[:],
                            rhs=gxT[t].rearrange("p (q c) -> p q c", q=4)[:, :, 32 * s:32 * (s + 1)],
                            start=(t != 1), stop=False)
                    # interleaved logits matmuls: also h(s)-independent tail filler
                    emitted = 0
                    while (emitted < 4 and s0_state["i"] < len(s0_jobs)
                           and s0_jobs[s0_state["i"]][1] < s // 4):
                        super0_next()
                        emitted += 1
                    # drip phase-3 prefetch pieces onto the SP queue
                    if s >= 4:
                        for _ in range(2):
                            if piece_i[0] < len(pieces):
                                dst, src = pieces[piece_i[0]]
                                piece_i[0] += 1
                                nc.sync.dma_start(out=dst, in_=src)
                    # W_hh @ h, order g, i, f, o; k outer so low-k matmuls can
                    # start off the first half of h
                    for t in GATE_ORDER:
                        for k in range(4):
                            for q in range(4):
                                if s == 0:
                                    rh = hT0[:, BL * k:BL * (k + 1)]
                                else:
                                    rh = hall_b[:, M_TOK * k + BL * (s - 1): M_TOK * k + BL * s]
                                nc.tensor.matmul(
                                    out=tile_of[t][:, col0[t] + 32 * q:col0[t] + 32 * (q + 1)],
                                    lhsT=whh_sb[:, G4 * k + 512 * t + 128 * q:
                                                G4 * k + 512 * t + 128 * (q + 1)],
                                    rhs=rh, start=False,
                                    stop=(q == 3 and k == 3 and t != 0))
                        if t == 2:
                            nc.scalar.activation(out=actg[:], in_=pgg[:], func=TANH)
                        elif t == 1:
                            nc.scalar.activation(out=actif[:], in_=pgif[:], func=SIG)
                    # cell update: t1 on Pool, t2 on DVE
                    nc.gpsimd.tensor_tensor(out=t1_sb[:], in0=actif[:, 0:128], in1=actg[:], op=MUL)
                    nc.vector.tensor_tensor(out=t2_sb[:], in0=actif[:, 128:256], in1=cT[:], op=MUL)
                    nc.vector.tensor_tensor(out=cT[:], in0=t1_sb[:], in1=t2_sb[:], op=ADD)
                    # th in halves: th_lo unblocks h_lo (k0-1) a hop earlier
                    nc.scalar.activation(out=th_sb[:, 0:64], in_=cT[:, 0:64], func=TANH)
                    nc.scalar.activation(out=th_sb[:, 64:128], in_=cT[:, 64:128], func=TANH)
                    nc.scalar.activation(out=acto[:], in_=pgo[:], func=SIG)
                    # h (bf16 first, in halves: k0-1 unblocks the next step's
                    # low-k matmuls while k2-3 is still being written)
                    nc.vector.tensor_tensor(out=hall_b4[:, 0:2, BL * s:BL * (s + 1)],
                                            in0=acto4[:, 0:2], in1=th4[:, 0:2], op=MUL)
                    nc.vector.tensor_tensor(out=hall_b4[:, 2:4, BL * s:BL * (s + 1)],
                                            in0=acto4[:, 2:4], in1=th4[:, 2:4], op=MUL)
                    nc.gpsimd.tensor_tensor(out=hall_f4[:, :, BL * s:BL * (s + 1)],
                                            in0=acto4[:], in1=th4[:], op=MUL)
                # drain remaining super jobs and prefetch pieces
                while s0_state["i"] < len(s0_jobs):
                    super0_next()
                while piece_i[0] < len(pieces):
                    dst, src = pieces[piece_i[0]]
                    piece_i[0] += 1
                    nc.sync.dma_start(out=dst, in_=src)

            sst.release()
            spp.release()
            defp.release()
            gxp.release()

            # ---------- phase 3: logits cols V0:10000, fp32r ----------
            with tc.tile_pool(name="p3st", bufs=6) as p3st, \
                 tc.tile_pool(name="p3ps", bufs=2, space="PSUM") as p3ps:
                nst = 0
                # mt7's cols 0:V0 (its h finishes only at step 30) overlap the
                # first wl stream here instead of serializing after the loop
                mt, rows = NMT - 1, M_TOK - 128 * (NMT - 1)
                for sup in range(V0 // 512):
                    pl = p3ps.tile([128, W3], f32, tag="pl", name=f"pl7_{sup}")
                    for k in range(4):
                        nc.tensor.matmul(
                            out=pl[0:rows, 0:512],
                            lhsT=hall_f[:, M_TOK * k + 128 * mt: M_TOK * k + 128 * mt + rows],
                            rhs=wl0[:, V0 * k + 512 * sup: V0 * k + 512 * (sup + 1)],
                            start=(k == 0), stop=(k == 3))
                    stg = p3st.tile([128, W3], f32, tag="stg", name=f"stg7_{sup}")
                    nc.vector.tensor_tensor(out=stg[0:rows, 0:512], in0=pl[0:rows, 0:512],
                                            in1=blin0[0:rows, 512 * sup:512 * (sup + 1)], op=ADD)
                    eng = nc.sync if nst % 2 == 0 else nc.gpsimd
                    nst += 1
                    eng.dma_start(out=out_d[128 * mt:128 * mt + rows, 512 * sup:512 * (sup + 1)],
                                  in_=stg[0:rows, 0:512])
                for ns, (c0, c1) in enumerate(sup_bounds):
                    w_sup = c1 - c0
                    chunks = []
                    off = 0
                    while off < w_sup:
                        chunks.append((off, min(512, w_sup - off)))
                        off += 512
                    wl = wl3_ring[ns]
                    bl = bl3_ring[ns]
                    for m in range(NMT):
                        if m == 1 and ns + 2 < NS3:
                            wl3_ring.append(load_wl(ns + 2, nc.scalar))
                            bl3_ring.append(load_blin(ns + 2, nc.scalar))
                        rows = min(128, M_TOK - 128 * m)
                        pl = p3ps.tile([128, W3], f32, tag="pl")
                        for off, width in chunks:
                            for k in range(4):
                                nc.tensor.matmul(
                                    out=pl[0:rows, off:off + width],
                                    lhsT=hall_f[:, M_TOK * k + 128 * m: M_TOK * k + 128 * m + rows],
                                    rhs=wl[:, W3 * k + off: W3 * k + off + width],
                                    start=(k == 0), stop=(k == 3))
                        stg = p3st.tile([128, W3], f32, tag="stg")
                        nc.vector.tensor_tensor(out=stg[0:rows, 0:w_sup], in0=pl[0:rows, 0:w_sup],
                                                in1=bl[0:rows, 0:w_sup], op=ADD)
                        # rotate store queues (ACT joins once wl loads end)
                        if ns >= NS3 - 2:
                            eng = (nc.sync, nc.gpsimd, nc.scalar)[nst % 3]
                        else:
                            eng = nc.sync if nst % 2 == 0 else nc.gpsimd
                        nst += 1
                        eng.dma_start(out=out_d[128 * m:128 * m + rows, c0:c1],
                                      in_=stg[0:rows, 0:w_sup])

            wlp.release()
            p3b.release()
            p3w.release()

    nc.compile()
    return nc


def _prep_host(caps, latent, embed, W_ih, W_hh, b_ih, b_hh, W_lin, b_lin):
    import ml_dtypes
    bf = ml_dtypes.bfloat16
    caps = np.asarray(caps).astype(np.int32)
    latent = np.asarray(latent, dtype=np.float32)
    embed = np.ascontiguousarray(np.asarray(embed, dtype=np.float32))
    wihT = np.ascontiguousarray(np.asarray(W_ih, dtype=np.float32).T.astype(bf))  # [E, 4H]
    whhT = np.ascontiguousarray(np.asarray(W_hh, dtype=np.float32).T.astype(bf))  # [H, 4H]
    bias = (np.asarray(b_ih, dtype=np.float32) + np.asarray(b_hh, dtype=np.float32))
    # biasq[p, c] = bias[(c//128)*128 + p]
    blk = bias.reshape(G4 // 128, 128)            # [16, 128]
    biasq = np.ascontiguousarray(
        np.broadcast_to(blk.T[:, :, None], (128, G4 // 128, 128))
        .reshape(128, G4).astype(bf))
    wlinT = np.ascontiguousarray(np.asarray(W_lin, dtype=np.float32).T)   # [H, V]
    blinb = np.ascontiguousarray(np.broadcast_to(
        np.asarray(b_lin, dtype=np.float32)[None, :], (128, V)).astype(bf))
    id128 = np.eye(128, dtype=np.float32)
    id128b = np.eye(128).astype(bf)

    in_maps = []
    for c in range(NCORES):
        caps_sh = caps[c * BL:(c + 1) * BL]                   # [32, 32]
        tok_flat = caps_sh[:, :S].T.reshape(M_TOK)            # t-major [992]
        tok_pad = np.zeros(NMT * 128, dtype=np.int32)
        tok_pad[:M_TOK] = tok_flat
        tok = np.ascontiguousarray(tok_pad.reshape(NMT, 128).T)  # [128, NMT]
        in_maps.append(dict(
            emb=embed, wihT=wihT, whhT=whhT, biasq=biasq, wlinT=wlinT,
            blinb=blinb, tok=tok, lat=np.ascontiguousarray(latent[c * BL:(c + 1) * BL]),
            id128=id128, id128b=id128b,
        ))
    return in_maps


def kernel(caps, latent, embed, W_ih, W_hh, b_ih, b_hh, W_lin, b_lin):
    from concourse.bass_utils import run_bass_kernel_spmd

    if "nc" not in _CACHE:
        _CACHE["nc"] = _build()
    nc = _CACHE["nc"]

    in_maps = _prep_host(caps, latent, embed, W_ih, W_hh, b_ih, b_hh, W_lin, b_lin)
    res = run_bass_kernel_spmd(nc, in_maps, core_ids=list(range(NCORES)))
    out = np.zeros((T, B_FULL, V), dtype=np.float32)
    for c in range(NCORES):
        shard = res.results[c]["out"].reshape(S, BL, V)
        out[1:, c * BL:(c + 1) * BL, :] = shard
    return out


# revision 78
# speedup vs baseline: 2.5041x; 1.0171x over previous
"""Teacher-forced decoder LSTM on 8 TRN2 NeuronCores.

Problem: B=256, T=32, V=10000, E=H=512 (fp32).
  step s in 0..30: x = embed[caps[:, s]]
                   gates = x@W_ih.T + h@W_hh.T + b     (i,f,g,o)
                   c = sig(f)*c + sig(i)*tanh(g); h = sig(o)*tanh(c)
                   out[s+1] = h@W_lin.T + b_lin
  out[0] = 0.  Output [T, B, V].

Sharding: data-parallel over batch, B_local=32 per core.

Layout: the recurrence runs fully TRANSPOSED (gate/hidden dims on
partitions, batch on the free axis) so each recurrent matmul moves only
32 columns. bf16 weights/activations in the gate path give 1 cyc/row on
the PE at any free size; the logits GEMM stays fp32r off the f32 copy of
h for accuracy.

  phase 1: gather X = embed[tok], PE-transpose, GxT = W_ih@X.T + b as
     bf16 tiles [128, (q)(tok)] per gate type (bias folded in).
  phase 2 (recurrent): per step 4 whole-bank gxT-inject matmuls (start
     the psum group) + 64 W_hh matmuls, all [*, 32/128]-moving bf16; ACT
     sig/tanh straight from PSUM; DVE/Pool cell update; h written
     directly into transposed history (bf16 for the recurrence, f32r for
     the logits GEMM) - no per-step transposes. Logits cols 0:1024 are
     interleaved into the PE stall windows between steps, and the first
     two phase-3 weight super-chunks prefetch on the idle SP DMA queue.
  phase 3: logits cols 1024:10000 as fp32r GEMM streamed per ~1800-col
     super-chunk, stores alternating SP/Pool DMA queues.
"""
import numpy as np

B_FULL, T, V, E, H = 256, 32, 10000, 512, 512
NCORES = 8
BL = B_FULL // NCORES          # 32 batch per core
S = T - 1                      # 31 recurrent steps
M_TOK = S * BL                 # 992 token rows per core
G4 = 4 * H                     # 2048 gate dims
NMT = (M_TOK + 127) // 128     # 8 token m-tiles (last is 96 rows)
V0 = 2048                      # vocab cols done inside phase 2
W3 = 1536                      # phase-3 super-chunk width (3 x 512)
NS3 = 6                        # phase-3 super count (5 x 1536 + 272)

_CACHE = {}


def _build():
    import concourse.bacc as bacc
    import concourse.mybir as mybir
    from concourse.tile import TileContext
    import concourse.bass as bass

    f32 = mybir.dt.float32
    f32r = mybir.dt.float32r
    bf16 = mybir.dt.bfloat16
    i32 = mybir.dt.int32
    SIG = mybir.ActivationFunctionType.Sigmoid
    TANH = mybir.ActivationFunctionType.Tanh
    ADD = mybir.AluOpType.add
    MUL = mybir.AluOpType.mult

    nc = bacc.Bacc()

    emb_d = nc.dram_tensor("emb", [V, E], f32r, kind="ExternalInput")
    wihT_d = nc.dram_tensor("wihT", [E, G4], bf16, kind="ExternalInput")
    whhT_d = nc.dram_tensor("whhT", [H, G4], bf16, kind="ExternalInput")
    biasq_d = nc.dram_tensor("biasq", [128, G4], bf16, kind="ExternalInput")
    wlinT_d = nc.dram_tensor("wlinT", [H, V], f32r, kind="ExternalInput")
    blinb_d = nc.dram_tensor("blinb", [128, V], bf16, kind="ExternalInput")
    tok_d = nc.dram_tensor("tok", [128, NMT], i32, kind="ExternalInput")
    lat_d = nc.dram_tensor("lat", [BL, H], f32r, kind="ExternalInput")
    id128_d = nc.dram_tensor("id128", [128, 128], f32r, kind="ExternalInput")
    id128b_d = nc.dram_tensor("id128b", [128, 128], bf16, kind="ExternalInput")
    out_d = nc.dram_tensor("out", [M_TOK, V], f32, kind="ExternalOutput")

    GATE_ORDER = (2, 0, 1, 3)   # g, i, f, o: start the tanh_g chain early

    with TileContext(nc) as tc:
        with tc.tile_pool(name="const", bufs=1) as cp, \
             tc.tile_pool(name="state", bufs=1) as st:

            # ---------- constants ----------
            tok_sb = cp.tile([128, NMT], i32, tag="tok_sb")
            nc.sync.dma_start(out=tok_sb[:], in_=tok_d[:])
            id128 = cp.tile([128, 128], f32r, tag="id128")
            nc.sync.dma_start(out=id128[:], in_=id128_d[:])
            lat_sb = cp.tile([BL, H], f32r, tag="lat_sb")
            nc.sync.dma_start(out=lat_sb[:], in_=lat_d[:])
            id128b = cp.tile([128, 128], bf16, tag="id128b")
            nc.sync.dma_start(out=id128b[:], in_=id128b_d[:])

            # ---------- state ----------
            hall_f = st.tile([128, 4 * M_TOK], f32r, tag="hall_f")
            hall_b = st.tile([128, 4 * M_TOK], bf16, tag="hall_b")
            hT0 = st.tile([128, 4 * BL], bf16, tag="hT0")   # col = 32k + b
            cT = st.tile([128, 4 * BL], f32, tag="cT")
            nc.vector.memset(cT[:], 0.0)
            actif = st.tile([128, 8 * BL], f32, tag="actif")
            actg = st.tile([128, 4 * BL], f32, tag="actg")
            acto = st.tile([128, 4 * BL], f32, tag="acto")
            t1_sb = st.tile([128, 4 * BL], f32, tag="t1_sb")
            t2_sb = st.tile([128, 4 * BL], f32, tag="t2_sb")
            th_sb = st.tile([128, 4 * BL], f32, tag="th_sb")
            # preload sigmoid/tanh activation tables during setup
            nc.scalar.activation(out=t1_sb[0:1, 0:1], in_=cT[0:1, 0:1], func=SIG)
            nc.scalar.activation(out=t2_sb[0:1, 0:1], in_=cT[0:1, 0:1], func=TANH)

            # phase-3 weight/bias rings (allocated first so they outlive gxp)
            p3w = tc.alloc_tile_pool(name="p3w", bufs=2)
            p3b = tc.alloc_tile_pool(name="p3b", bufs=3)
            sup_bounds = [(V0 + W3 * i, min(V0 + W3 * (i + 1), V)) for i in range(NS3)]

            def load_wl(ns, eng):
                c0, c1 = sup_bounds[ns]
                wl = p3w.tile([128, 4 * W3], f32r, tag="wl", name=f"wl{ns}")
                for k in range(4):
                    eng.dma_start(out=wl[:, W3 * k:W3 * k + (c1 - c0)],
                                  in_=wlinT_d[128 * k:128 * (k + 1), c0:c1])
                return wl

            def load_blin(ns, eng):
                c0, c1 = sup_bounds[ns]
                bl = p3b.tile([128, W3], bf16, tag="bl3", name=f"bl3_{ns}")
                eng.dma_start(out=bl[:, 0:c1 - c0], in_=blinb_d[:, c0:c1])
                return bl

            def wl_pieces(ns, wl):
                # split one wl super-load into small DMAs that drip onto the
                # SP queue between recurrence steps (big transfers would
                # block the interleaved logit stores behind them)
                c0, c1 = sup_bounds[ns]
                ps = []
                for k in range(4):
                    for h0 in range(0, c1 - c0, 512):
                        h1 = min(h0 + 512, c1 - c0)
                        ps.append((wl[:, W3 * k + h0:W3 * k + h1],
                                   wlinT_d[128 * k:128 * (k + 1), c0 + h0:c0 + h1]))
                return ps

            def blin_pieces(ns, bl):
                c0, c1 = sup_bounds[ns]
                ps = []
                for h0 in range(0, c1 - c0, 512):
                    h1 = min(h0 + 512, c1 - c0)
                    ps.append((bl[:, h0:h1], blinb_d[:, c0 + h0:c0 + h1]))
                return ps

            # super0 weights/bias live through phase 3 (mt7 jobs run there)
            wlp = tc.alloc_tile_pool(name="wlp", bufs=1)
            # long-lived recurrence inputs (released before phase 3)
            gxp = tc.alloc_tile_pool(name="gxp", bufs=1)
            whh_sb = gxp.tile([128, 4 * G4], bf16, tag="whh_sb")
            gxT = [gxp.tile([128, 4 * M_TOK], bf16, tag=f"gxT{t}", name=f"gxT{t}")
                   for t in range(4)]
            # super0 (vocab cols 0:V0) weights, interleaved into phase 2
            wl0 = wlp.tile([128, 4 * V0], f32r, tag="wl0")
            blin0 = wlp.tile([128, V0], bf16, tag="blin0")
            # tensors used by deferred phase-1 m-tiles (6,7), whose GEMMs run
            # inside the empty tails of recurrence steps 0..3
            defp = tc.alloc_tile_pool(name="defp", bufs=1)
            wih_sb = defp.tile([128, 4 * G4], bf16, tag="wih_sb")
            biasq = defp.tile([128, G4], bf16, tag="biasq")
            xt_def = [defp.tile([128, 512], bf16, tag=f"xtd{m}", name=f"xtd{m}")
                      for m in (6, 7)]

            spp = tc.alloc_tile_pool(name="spp", bufs=2, space="PSUM")
            sst = tc.alloc_tile_pool(name="sst", bufs=4)

            # ---------- phase 1: gather X, transpose, GxT = W_ih@X.T + b ----------
            with tc.tile_pool(name="p1", bufs=1) as p1, \
                 tc.tile_pool(name="p1ps", bufs=2, space="PSUM") as p1ps, \
                 tc.tile_pool(name="xps", bufs=2, space="PSUM") as xps, \
                 tc.tile_pool(name="tpsum", bufs=1, space="PSUM") as tps:
                # W_ih in 4 k-chunk DMAs so the first GEMM starts early;
                # wl0 after it on the same queue (not needed until step 4)
                for k in range(4):
                    eng = nc.scalar if k % 2 == 0 else nc.sync
                    eng.dma_start(out=wih_sb[:, G4 * k:G4 * (k + 1)],
                                  in_=wihT_d[128 * k:128 * (k + 1), :])
                nc.sync.dma_start(out=biasq[:], in_=biasq_d[:])
                # whh/blin0/wl0 after wih+biasq: not needed until step 0+
                nc.sync.dma_start(out=whh_sb[:].rearrange("p (k m) -> p k m", k=4),
                                  in_=whhT_d.rearrange("(k p) m -> p k m", k=4))
                nc.sync.dma_start(out=blin0[:], in_=blinb_d[:, 0:V0])
                for k in range(4):
                    nc.scalar.dma_start(out=wl0[:, V0 * k:V0 * (k + 1)],
                                        in_=wlinT_d[128 * k:128 * (k + 1), 0:V0])

                # transpose h0 = latent -> hT0 (one psum tile, one copy)
                pt0 = tps.tile([128, 128], f32r, tag="pt0")
                for k in range(4):
                    nc.tensor.transpose(out=pt0[0:128, BL * k:BL * (k + 1)],
                                        in_=lat_sb[:, 128 * k:128 * (k + 1)],
                                        identity=id128[0:BL, 0:BL])
                nc.vector.tensor_copy(out=hT0[:], in_=pt0[:])

                def gx_gemm(m, xt, pool, tag):
                    rows = min(128, M_TOK - 128 * m)
                    for t in range(4):
                        pg = pool.tile([128, 512], f32, tag=tag, name=f"pg1_{m}_{t}")
                        for q in range(4):
                            for k in range(4):
                                nc.tensor.matmul(
                                    out=pg[:, 128 * q:128 * q + rows],
                                    lhsT=wih_sb[:, G4 * k + 512 * t + 128 * q:
                                                G4 * k + 512 * t + 128 * (q + 1)],
                                    rhs=xt[:, 128 * k:128 * k + rows],
                                    start=(k == 0), stop=(k == 3))
                        # add bias, cast to bf16 (DVE: GPSIMD cannot read PSUM)
                        ai = nc.vector.tensor_tensor(
                            out=gxT[t].rearrange("p (q c) -> p q c", q=4)[:, :, 128 * m:128 * m + rows],
                            in0=pg.rearrange("p (q c) -> p q c", q=4)[:, :, 0:rows],
                            in1=biasq[:, 512 * t:512 * (t + 1)].rearrange(
                                "p (q c) -> p q c", q=4)[:, :, 0:rows],
                            op=ADD)
                        if m >= 6:
                            def_adds.append(ai)

                for m in range(NMT):
                    rows = min(128, M_TOK - 128 * m)
                    x_m = p1.tile([128, E], f32r, tag="x_m", bufs=2, name=f"x_m{m}")
                    nc.gpsimd.indirect_dma_start(
                        out=x_m[0:rows, :], out_offset=None, in_=emb_d[:],
                        in_offset=bass.IndirectOffsetOnAxis(ap=tok_sb[0:rows, m:m + 1], axis=0))
                    xp = xps.tile([128, 512], f32r, tag="xp", name=f"xp{m}")
                    for k in range(4):
                        nc.tensor.transpose(out=xp[0:128, 128 * k:128 * k + rows],
                                            in_=x_m[0:rows, 128 * k:128 * (k + 1)],
                                            identity=id128[0:rows, 0:rows])
                    xt = (p1.tile([128, 512], bf16, tag="xt", bufs=2, name=f"xt{m}")
                          if m < 6 else xt_def[m - 6])
                    nc.vector.tensor_copy(
                        out=xt.rearrange("p (k c) -> p k c", k=4)[:, :, 0:rows],
                        in_=xp.rearrange("p (k c) -> p k c", k=4)[:, :, 0:rows])
                    # m 6,7: GEMM deferred into the tails of steps 0..3
                    if m < 6:
                        gx_gemm(m, xt, p1ps, "pg1")

            # deferred m 6,7 GEMMs: deprioritized fillers for steps 0..3
            # (their gxT columns are not consumed until step 24)
            def_adds = []
            with tc.high_priority(offset=-600):
                gx_gemm(6, xt_def[0], spp, "pl0")
                gx_gemm(7, xt_def[1], spp, "pl0")

            # ---------- phase 2: recurrence with interleaved logits cols 0:V0 ----
            # super jobs: (sup, mt, k) matmuls + fin; emitted into PE stall windows
            s0_jobs = []
            for mt in range(NMT - 1):
                for sup in range(V0 // 512):
                    for k in range(4):
                        s0_jobs.append((sup, mt, k))
            s0_state = {"i": 0, "pl": None, "wl3": 0, "fins": []}

            def super0_next():
                sup, mt, k = s0_jobs[s0_state["i"]]
                s0_state["i"] += 1
                rows = min(128, M_TOK - 128 * mt)
                # deprioritized: fill PE/DVE idle slots, never delay the
                # recurrence chain (the scheduler would otherwise hoist these
                # between critical matmuls)
                with tc.high_priority(offset=-600):
                    if k == 0:
                        s0_state["pl"] = spp.tile([128, 512], f32, tag="pl0",
                                                  name=f"pl0_{sup}_{mt}")
                    pl = s0_state["pl"]
                    nc.tensor.matmul(
                        out=pl[0:rows, :],
                        lhsT=hall_f[:, M_TOK * k + 128 * mt: M_TOK * k + 128 * mt + rows],
                        rhs=wl0[:, V0 * k + 512 * sup: V0 * k + 512 * (sup + 1)],
                        start=(k == 0), stop=(k == 3))
                    if k == 3:
                        stg = sst.tile([128, 512], f32, tag="stg0", name=f"stg0_{sup}_{mt}")
                        # half-width adds; made dependent on the emitting
                        # step's final h-write afterwards, so they can never
                        # slot in front of it on the in-order DVE queue
                        for hh in (0, 256):
                            fi = nc.vector.tensor_tensor(
                                out=stg[0:rows, hh:hh + 256], in0=pl[0:rows, hh:hh + 256],
                                in1=blin0[0:rows, 512 * sup + hh:512 * sup + hh + 256], op=ADD)
                            s0_state["fins"].append(fi)
                        nc.sync.dma_start(
                            out=out_d[128 * mt:128 * mt + rows, 512 * sup:512 * (sup + 1)],
                            in_=stg[0:rows, :])

            hall_b4 = hall_b.rearrange("p (k c) -> p k c", k=4)
            hall_f4 = hall_f.rearrange("p (k c) -> p k c", k=4)
            acto4 = acto.rearrange("p (q b) -> p q b", q=4)
            th4 = th_sb.rearrange("p (q b) -> p q b", q=4)
            # prefetch pieces for phase-3 supers 0,1 (weights + bias), SP queue
            wl3_ring = [p3w.tile([128, 4 * W3], f32r, tag="wl", name="wl0"),
                        p3w.tile([128, 4 * W3], f32r, tag="wl", name="wl1")]
            bl3_ring = [p3b.tile([128, W3], bf16, tag="bl3", name="bl3_0"),
                        p3b.tile([128, W3], bf16, tag="bl3", name="bl3_1")]
            pieces = (wl_pieces(0, wl3_ring[0]) + blin_pieces(0, bl3_ring[0])
                      + wl_pieces(1, wl3_ring[1]) + blin_pieces(1, bl3_ring[1]))
            piece_i = [0]

            with tc.tile_pool(name="rps", bufs=2, space="PSUM") as rps:
                for s in range(S):
                    # i and f share one psum bank (one group, one sig_if ACT)
                    pgif = rps.tile([128, 256], f32, tag="pgif", name=f"pgif_{s}")
                    pgg = rps.tile([128, 128], f32, tag="pgg", name=f"pgg_{s}")
                    pgo = rps.tile([128, 128], f32, tag="pgo", name=f"pgo_{s}")
                    tile_of = {0: pgif, 1: pgif, 2: pgg, 3: pgo}
                    col0 = {0: 0, 1: 128, 2: 0, 3: 0}
                    # whole-bank gxT injects: start each psum group.
                    # Independent of h -> run inside the previous step's tail.
                    for t in GATE_ORDER:
                        nc.tensor.matmul(
                            out=tile_of[t][:, col0[t]:col0[t] + 128],
                            lhsT=id128b[:],
                            rhs=gxT[t].rearrange("p (q c) -> p q c", q=4)[:, :, 32 * s:32 * (s + 1)],
                            start=(t != 1), stop=False)
                    # interleaved logits matmuls: also h(s)-independent tail filler
                    emitted = 0
                    while (emitted < 4 and s0_state["i"] < len(s0_jobs)
                           and s0_jobs[s0_state["i"]][1] < s // 4):
                        super0_next()
                        emitted += 1
                    # drip phase-3 prefetch pieces onto the SP queue
                    if s >= 4:
                        for _ in range(2):
                            if piece_i[0] < len(pieces):
                                dst, src = pieces[piece_i[0]]
                                piece_i[0] += 1
                                nc.sync.dma_start(out=dst, in_=src)
                    # W_hh @ h, order g, i, f, o; k outer so low-k matmuls can
                    # start off the first half of h
                    for t in GATE_ORDER:
                        for k in range(4):
                            for q in range(4):
                                if s == 0:
                                    rh = hT0[:, BL * k:BL * (k + 1)]
                                else:
                                    rh = hall_b[:, M_TOK * k + BL * (s - 1): M_TOK * k + BL * s]
                                nc.tensor.matmul(
                                    out=tile_of[t][:, col0[t] + 32 * q:col0[t] + 32 * (q + 1)],
                                    lhsT=whh_sb[:, G4 * k + 512 * t + 128 * q:
                                                G4 * k + 512 * t + 128 * (q + 1)],
                                    rhs=rh, start=False,
                                    stop=(q == 3 and k == 3 and t != 0))
                        if t == 2:
                            nc.scalar.activation(out=actg[:], in_=pgg[:], func=TANH)
                        elif t == 1:
                            nc.scalar.activation(out=actif[:], in_=pgif[:], func=SIG)
                    # cell update: t1 on Pool, t2 on DVE
                    nc.gpsimd.tensor_tensor(out=t1_sb[:], in0=actif[:, 0:128], in1=actg[:], op=MUL)
                    nc.vector.tensor_tensor(out=t2_sb[:], in0=actif[:, 128:256], in1=cT[:], op=MUL)
                    nc.vector.tensor_tensor(out=cT[:], in0=t1_sb[:], in1=t2_sb[:], op=ADD)
                    # th in halves: th_lo unblocks h_lo (k0-1) a hop earlier
                    nc.scalar.activation(out=th_sb[:, 0:64], in_=cT[:, 0:64], func=TANH)
                    nc.scalar.activation(out=th_sb[:, 64:128], in_=cT[:, 64:128], func=TANH)
                    nc.scalar.activation(out=acto[:], in_=pgo[:], func=SIG)
                    # h (bf16 first, in halves: k0-1 unblocks the next step's
                    # low-k matmuls while k2-3 is still being written)
                    nc.vector.tensor_tensor(out=hall_b4[:, 0:2, BL * s:BL * (s + 1)],
                                            in0=acto4[:, 0:2], in1=th4[:, 0:2], op=MUL)
                    hb_i = nc.vector.tensor_tensor(out=hall_b4[:, 2:4, BL * s:BL * (s + 1)],
                                                   in0=acto4[:, 2:4], in1=th4[:, 2:4], op=MUL)
                    nc.gpsimd.tensor_tensor(out=hall_f4[:, :, BL * s:BL * (s + 1)],
                                            in0=acto4[:], in1=th4[:], op=MUL)
                    import bass_rust as _br
                    _DI = _br.DependencyInfo(sync=True, no_sync=False)
                    for fi in s0_state["fins"]:
                        fi.ins.add_dependency(hb_i.ins.name, _DI)
                    s0_state["fins"] = []
                    for fi in def_adds[:2]:
                        fi.ins.add_dependency(hb_i.ins.name, _DI)
                    del def_adds[:2]
                # drain remaining super jobs and prefetch pieces
                while s0_state["i"] < len(s0_jobs):
                    super0_next()
                while piece_i[0] < len(pieces):
                    dst, src = pieces[piece_i[0]]
                    piece_i[0] += 1
                    nc.sync.dma_start(out=dst, in_=src)

            sst.release()
            spp.release()
            defp.release()
            gxp.release()

            # ---------- phase 3: logits cols V0:10000, fp32r ----------
            with tc.tile_pool(name="p3st", bufs=6) as p3st, \
                 tc.tile_pool(name="p3ps", bufs=2, space="PSUM") as p3ps:
                nst = 0
                # mt7's cols 0:V0 (its h finishes only at step 30) overlap the
                # first wl stream here instead of serializing after the loop
                mt, rows = NMT - 1, M_TOK - 128 * (NMT - 1)
                for sup in range(V0 // 512):
                    pl = p3ps.tile([128, W3], f32, tag="pl", name=f"pl7_{sup}")
                    for k in range(4):
                        nc.tensor.matmul(
                            out=pl[0:rows, 0:512],
                            lhsT=hall_f[:, M_TOK * k + 128 * mt: M_TOK * k + 128 * mt + rows],
                            rhs=wl0[:, V0 * k + 512 * sup: V0 * k + 512 * (sup + 1)],
                            start=(k == 0), stop=(k == 3))
                    stg = p3st.tile([128, W3], f32, tag="stg", name=f"stg7_{sup}")
                    nc.vector.tensor_tensor(out=stg[0:rows, 0:512], in0=pl[0:rows, 0:512],
                                            in1=blin0[0:rows, 512 * sup:512 * (sup + 1)], op=ADD)
                    eng = nc.sync if nst % 2 == 0 else nc.gpsimd
                    nst += 1
                    eng.dma_start(out=out_d[128 * mt:128 * mt + rows, 512 * sup:512 * (sup + 1)],
                                  in_=stg[0:rows, 0:512])
                for ns, (c0, c1) in enumerate(sup_bounds):
                    w_sup = c1 - c0
                    chunks = []
                    off = 0
                    while off < w_sup:
                        chunks.append((off, min(512, w_sup - off)))
                        off += 512
                    wl = wl3_ring[ns]
                    bl = bl3_ring[ns]
                    for m in range(NMT):
                        if m == 1 and ns + 2 < NS3:
                            wl3_ring.append(load_wl(ns + 2, nc.scalar))
                            bl3_ring.append(load_blin(ns + 2, nc.scalar))
                        rows = min(128, M_TOK - 128 * m)
                        pl = p3ps.tile([128, W3], f32, tag="pl")
                        for off, width in chunks:
                            for k in range(4):
                                nc.tensor.matmul(
                                    out=pl[0:rows, off:off + width],
                                    lhsT=hall_f[:, M_TOK * k + 128 * m: M_TOK * k + 128 * m + rows],
                                    rhs=wl[:, W3 * k + off: W3 * k + off + width],
                                    start=(k == 0), stop=(k == 3))
                        stg = p3st.tile([128, W3], f32, tag="stg")
                        nc.vector.tensor_tensor(out=stg[0:rows, 0:w_sup], in0=pl[0:rows, 0:w_sup],
                                                in1=bl[0:rows, 0:w_sup], op=ADD)
                        # rotate store queues (ACT joins once wl loads end)
                        if ns >= NS3 - 2:
                            eng = (nc.sync, nc.gpsimd, nc.scalar)[nst % 3]
                        else:
                            eng = nc.sync if nst % 2 == 0 else nc.gpsimd
                        nst += 1
                        eng.dma_start(out=out_d[128 * m:128 * m + rows, c0:c1],
                                      in_=stg[0:rows, 0:w_sup])

            wlp.release()
            p3b.release()
            p3w.release()

    nc.compile()
    return nc


def _prep_host(caps, latent, embed, W_ih, W_hh, b_ih, b_hh, W_lin, b_lin):
    import ml_dtypes
    bf = ml_dtypes.bfloat16
    caps = np.asarray(caps).astype(np.int32)
    latent = np.asarray(latent, dtype=np.float32)
    embed = np.ascontiguousarray(np.asarray(embed, dtype=np.float32))
    wihT = np.ascontiguousarray(np.asarray(W_ih, dtype=np.float32).T.astype(bf))  # [E, 4H]
    whhT = np.ascontiguousarray(np.asarray(W_hh, dtype=np.float32).T.astype(bf))  # [H, 4H]
    bias = (np.asarray(b_ih, dtype=np.float32) + np.asarray(b_hh, dtype=np.float32))
    # biasq[p, c] = bias[(c//128)*128 + p]
    blk = bias.reshape(G4 // 128, 128)            # [16, 128]
    biasq = np.ascontiguousarray(
        np.broadcast_to(blk.T[:, :, None], (128, G4 // 128, 128))
        .reshape(128, G4).astype(bf))
    wlinT = np.ascontiguousarray(np.asarray(W_lin, dtype=np.float32).T)   # [H, V]
    blinb = np.ascontiguousarray(np.broadcast_to(
        np.asarray(b_lin, dtype=np.float32)[None, :], (128, V)).astype(bf))
    id128 = np.eye(128, dtype=np.float32)
    id128b = np.eye(128).astype(bf)

    in_maps = []
    for c in range(NCORES):
        caps_sh = caps[c * BL:(c + 1) * BL]                   # [32, 32]
        tok_flat = caps_sh[:, :S].T.reshape(M_TOK)            # t-major [992]
        tok_pad = np.zeros(NMT * 128, dtype=np.int32)
        tok_pad[:M_TOK] = tok_flat
        tok = np.ascontiguousarray(tok_pad.reshape(NMT, 128).T)  # [128, NMT]
        in_maps.append(dict(
            emb=embed, wihT=wihT, whhT=whhT, biasq=biasq, wlinT=wlinT,
            blinb=blinb, tok=tok, lat=np.ascontiguousarray(latent[c * BL:(c + 1) * BL]),
            id128=id128, id128b=id128b,
        ))
    return in_maps


def kernel(caps, latent, embed, W_ih, W_hh, b_ih, b_hh, W_lin, b_lin):
    from concourse.bass_utils import run_bass_kernel_spmd

    if "nc" not in _CACHE:
        _CACHE["nc"] = _build()
    nc = _CACHE["nc"]

    in_maps = _prep_host(caps, latent, embed, W_ih, W_hh, b_ih, b_hh, W_lin, b_lin)
    res = run_bass_kernel_spmd(nc, in_maps, core_ids=list(range(NCORES)))
    out = np.zeros((T, B_FULL, V), dtype=np.float32)
    for c in range(NCORES):
        shard = res.results[c]["out"].reshape(S, BL, V)
        out[1:, c * BL:(c + 1) * BL, :] = shard
    return out


# revision 86
# speedup vs baseline: 2.5618x; 1.0230x over previous
"""Teacher-forced decoder LSTM on 8 TRN2 NeuronCores.

Problem: B=256, T=32, V=10000, E=H=512 (fp32).
  step s in 0..30: x = embed[caps[:, s]]
                   gates = x@W_ih.T + h@W_hh.T + b     (i,f,g,o)
                   c = sig(f)*c + sig(i)*tanh(g); h = sig(o)*tanh(c)
                   out[s+1] = h@W_lin.T + b_lin
  out[0] = 0.  Output [T, B, V].

Sharding: data-parallel over batch, B_local=32 per core.

Layout: the recurrence runs fully TRANSPOSED (gate/hidden dims on
partitions, batch on the free axis) so each recurrent matmul moves only
32 columns. bf16 weights/activations in the gate path give 1 cyc/row on
the PE at any free size; the logits GEMM stays fp32r off the f32 copy of
h for accuracy.

  phase 1: gather X = embed[tok], PE-transpose, GxT = W_ih@X.T + b as
     bf16 tiles [128, (q)(tok)] per gate type (bias folded in).
  phase 2 (recurrent): per step 4 whole-bank gxT-inject matmuls (start
     the psum group) + 64 W_hh matmuls, all [*, 32/128]-moving bf16; ACT
     sig/tanh straight from PSUM; DVE/Pool cell update; h written
     directly into transposed history (bf16 for the recurrence, f32r for
     the logits GEMM) - no per-step transposes. Logits cols 0:1024 are
     interleaved into the PE stall windows between steps, and the first
     two phase-3 weight super-chunks prefetch on the idle SP DMA queue.
  phase 3: logits cols 1024:10000 as fp32r GEMM streamed per ~1800-col
     super-chunk, stores alternating SP/Pool DMA queues.
"""
import numpy as np

B_FULL, T, V, E, H = 256, 32, 10000, 512, 512
NCORES = 8
BL = B_FULL // NCORES          # 32 batch per core
S = T - 1                      # 31 recurrent steps
M_TOK = S * BL                 # 992 token rows per core
G4 = 4 * H                     # 2048 gate dims
NMT = (M_TOK + 127) // 128     # 8 token m-tiles (last is 96 rows)
V0 = 2048                      # vocab cols done inside phase 2
W3 = 1536                      # phase-3 super-chunk width (3 x 512)
NS3 = 6                        # phase-3 super count (5 x 1536 + 272)

_CACHE = {}


def _build():
    import concourse.bacc as bacc
    import concourse.mybir as mybir
    from concourse.tile import TileContext
    import concourse.bass as bass

    f32 = mybir.dt.float32
    f32r = mybir.dt.float32r
    bf16 = mybir.dt.bfloat16
    i32 = mybir.dt.int32
    SIG = mybir.ActivationFunctionType.Sigmoid
    TANH = mybir.ActivationFunctionType.Tanh
    ADD = mybir.AluOpType.add
    MUL = mybir.AluOpType.mult

    nc = bacc.Bacc()

    emb_d = nc.dram_tensor("emb", [V, E], bf16, kind="ExternalInput")
    wihT_d = nc.dram_tensor("wihT", [E, G4], bf16, kind="ExternalInput")
    whhT_d = nc.dram_tensor("whhT", [H, G4], bf16, kind="ExternalInput")
    biasq_d = nc.dram_tensor("biasq", [128, G4], bf16, kind="ExternalInput")
    wlinT_d = nc.dram_tensor("wlinT", [H, V], f32r, kind="ExternalInput")
    blinb_d = nc.dram_tensor("blinb", [128, V], bf16, kind="ExternalInput")
    tok_d = nc.dram_tensor("tok", [128, NMT], i32, kind="ExternalInput")
    lat_d = nc.dram_tensor("lat", [BL, H], f32r, kind="ExternalInput")
    id128_d = nc.dram_tensor("id128", [128, 128], f32r, kind="ExternalInput")
    id128b_d = nc.dram_tensor("id128b", [128, 128], bf16, kind="ExternalInput")
    out_d = nc.dram_tensor("out", [M_TOK, V], f32, kind="ExternalOutput")

    GATE_ORDER = (2, 0, 1, 3)   # g, i, f, o: start the tanh_g chain early

    with TileContext(nc) as tc:
        with tc.tile_pool(name="const", bufs=1) as cp, \
             tc.tile_pool(name="state", bufs=1) as st:

            # ---------- constants ----------
            tok_sb = cp.tile([128, NMT], i32, tag="tok_sb")
            nc.sync.dma_start(out=tok_sb[:], in_=tok_d[:])
            id128 = cp.tile([128, 128], f32r, tag="id128")
            nc.sync.dma_start(out=id128[:], in_=id128_d[:])
            lat_sb = cp.tile([BL, H], f32r, tag="lat_sb")
            nc.sync.dma_start(out=lat_sb[:], in_=lat_d[:])
            id128b = cp.tile([128, 128], bf16, tag="id128b")
            nc.sync.dma_start(out=id128b[:], in_=id128b_d[:])

            # ---------- state ----------
            hall_f = st.tile([128, 4 * M_TOK], f32r, tag="hall_f")
            hall_b = st.tile([128, 4 * M_TOK], bf16, tag="hall_b")
            hT0 = st.tile([128, 4 * BL], bf16, tag="hT0")   # col = 32k + b
            cT = st.tile([128, 4 * BL], f32, tag="cT")
            nc.vector.memset(cT[:], 0.0)
            actif = st.tile([128, 8 * BL], f32, tag="actif")
            actg = st.tile([128, 4 * BL], f32, tag="actg")
            acto = st.tile([128, 4 * BL], f32, tag="acto")
            t1_sb = st.tile([128, 4 * BL], f32, tag="t1_sb")
            t2_sb = st.tile([128, 4 * BL], f32, tag="t2_sb")
            th_sb = st.tile([128, 4 * BL], f32, tag="th_sb")
            # preload sigmoid/tanh activation tables during setup
            nc.scalar.activation(out=t1_sb[0:1, 0:1], in_=cT[0:1, 0:1], func=SIG)
            nc.scalar.activation(out=t2_sb[0:1, 0:1], in_=cT[0:1, 0:1], func=TANH)

            # phase-3 weight/bias rings (allocated first so they outlive gxp)
            p3w = tc.alloc_tile_pool(name="p3w", bufs=2)
            p3b = tc.alloc_tile_pool(name="p3b", bufs=3)
            sup_bounds = [(V0 + W3 * i, min(V0 + W3 * (i + 1), V)) for i in range(NS3)]

            def load_wl(ns, eng):
                c0, c1 = sup_bounds[ns]
                wl = p3w.tile([128, 4 * W3], f32r, tag="wl", name=f"wl{ns}")
                for k in range(4):
                    eng.dma_start(out=wl[:, W3 * k:W3 * k + (c1 - c0)],
                                  in_=wlinT_d[128 * k:128 * (k + 1), c0:c1])
                return wl

            def load_blin(ns, eng):
                c0, c1 = sup_bounds[ns]
                bl = p3b.tile([128, W3], bf16, tag="bl3", name=f"bl3_{ns}")
                eng.dma_start(out=bl[:, 0:c1 - c0], in_=blinb_d[:, c0:c1])
                return bl

            def wl_pieces(ns, wl):
                # split one wl super-load into small DMAs that drip onto the
                # SP queue between recurrence steps (big transfers would
                # block the interleaved logit stores behind them)
                c0, c1 = sup_bounds[ns]
                ps = []
                for k in range(4):
                    for h0 in range(0, c1 - c0, 512):
                        h1 = min(h0 + 512, c1 - c0)
                        ps.append((wl[:, W3 * k + h0:W3 * k + h1],
                                   wlinT_d[128 * k:128 * (k + 1), c0 + h0:c0 + h1]))
                return ps

            def blin_pieces(ns, bl):
                c0, c1 = sup_bounds[ns]
                ps = []
                for h0 in range(0, c1 - c0, 512):
                    h1 = min(h0 + 512, c1 - c0)
                    ps.append((bl[:, h0:h1], blinb_d[:, c0 + h0:c0 + h1]))
                return ps

            # super0 weights/bias live through phase 3 (mt7 jobs run there)
            wlp = tc.alloc_tile_pool(name="wlp", bufs=1)
            # long-lived recurrence inputs (released before phase 3)
            gxp = tc.alloc_tile_pool(name="gxp", bufs=1)
            whh_sb = gxp.tile([128, 4 * G4], bf16, tag="whh_sb")
            gxT = [gxp.tile([128, 4 * M_TOK], bf16, tag=f"gxT{t}", name=f"gxT{t}")
                   for t in range(4)]
            # super0 (vocab cols 0:V0) weights, interleaved into phase 2
            wl0 = wlp.tile([128, 4 * V0], f32r, tag="wl0")
            blin0 = wlp.tile([128, V0], bf16, tag="blin0")
            # tensors used by deferred phase-1 m-tiles (6,7), whose GEMMs run
            # inside the empty tails of recurrence steps 0..3
            defp = tc.alloc_tile_pool(name="defp", bufs=1)
            wih_sb = defp.tile([128, 4 * G4], bf16, tag="wih_sb")
            biasq = defp.tile([128, G4], bf16, tag="biasq")
            xt_def = [defp.tile([128, 512], bf16, tag=f"xtd{m}", name=f"xtd{m}")
                      for m in (2, 3, 4, 5, 6, 7)]

            spp = tc.alloc_tile_pool(name="spp", bufs=2, space="PSUM")
            sst = tc.alloc_tile_pool(name="sst", bufs=3)

            # ---------- phase 1: gather X, transpose, GxT = W_ih@X.T + b ----------
            with tc.tile_pool(name="p1", bufs=1) as p1, \
                 tc.tile_pool(name="p1ps", bufs=2, space="PSUM") as p1ps, \
                 tc.tile_pool(name="xps", bufs=2, space="PSUM") as xps, \
                 tc.tile_pool(name="tpsum", bufs=1, space="PSUM") as tps:
                # W_ih in 4 k-chunk DMAs so the first GEMM starts early;
                # wl0 after it on the same queue (not needed until step 4)
                for k in range(4):
                    eng = nc.scalar if k % 2 == 0 else nc.sync
                    eng.dma_start(out=wih_sb[:, G4 * k:G4 * (k + 1)],
                                  in_=wihT_d[128 * k:128 * (k + 1), :])
                nc.sync.dma_start(out=biasq[:], in_=biasq_d[:])
                # whh/blin0/wl0 after wih+biasq: not needed until step 0+
                nc.sync.dma_start(out=whh_sb[:].rearrange("p (k m) -> p k m", k=4),
                                  in_=whhT_d.rearrange("(k p) m -> p k m", k=4))
                nc.sync.dma_start(out=blin0[:], in_=blinb_d[:, 0:V0])
                for k in range(4):
                    nc.scalar.dma_start(out=wl0[:, V0 * k:V0 * (k + 1)],
                                        in_=wlinT_d[128 * k:128 * (k + 1), 0:V0])

                # transpose h0 = latent -> hT0 (one psum tile, one copy)
                pt0 = tps.tile([128, 128], f32r, tag="pt0")
                for k in range(4):
                    nc.tensor.transpose(out=pt0[0:128, BL * k:BL * (k + 1)],
                                        in_=lat_sb[:, 128 * k:128 * (k + 1)],
                                        identity=id128[0:BL, 0:BL])
                nc.vector.tensor_copy(out=hT0[:], in_=pt0[:])

                def gx_gemm(m, xt, pool, tag):
                    rows = min(128, M_TOK - 128 * m)
                    for t in range(4):
                        pg = pool.tile([128, 512], f32, tag=tag, name=f"pg1_{m}_{t}")
                        for q in range(4):
                            for k in range(4):
                                nc.tensor.matmul(
                                    out=pg[:, 128 * q:128 * q + rows],
                                    lhsT=wih_sb[:, G4 * k + 512 * t + 128 * q:
                                                G4 * k + 512 * t + 128 * (q + 1)],
                                    rhs=xt[:, 128 * k:128 * k + rows],
                                    start=(k == 0), stop=(k == 3))
                        # add bias, cast to bf16 (DVE: GPSIMD cannot read PSUM)
                        ai = nc.vector.tensor_tensor(
                            out=gxT[t].rearrange("p (q c) -> p q c", q=4)[:, :, 128 * m:128 * m + rows],
                            in0=pg.rearrange("p (q c) -> p q c", q=4)[:, :, 0:rows],
                            in1=biasq[:, 512 * t:512 * (t + 1)].rearrange(
                                "p (q c) -> p q c", q=4)[:, :, 0:rows],
                            op=ADD)
                        if m >= 2:
                            def_adds.append(ai)

                for m in range(NMT):
                    rows = min(128, M_TOK - 128 * m)
                    x_m = p1.tile([128, E], bf16, tag="x_m", bufs=2, name=f"x_m{m}")
                    nc.gpsimd.indirect_dma_start(
                        out=x_m[0:rows, :], out_offset=None, in_=emb_d[:],
                        in_offset=bass.IndirectOffsetOnAxis(ap=tok_sb[0:rows, m:m + 1], axis=0))
                    xp = xps.tile([128, 512], bf16, tag="xp", name=f"xp{m}")
                    for k in range(4):
                        nc.tensor.transpose(out=xp[0:128, 128 * k:128 * k + rows],
                                            in_=x_m[0:rows, 128 * k:128 * (k + 1)],
                                            identity=id128b[0:rows, 0:rows])
                    xt = (p1.tile([128, 512], bf16, tag="xt", bufs=2, name=f"xt{m}")
                          if m < 2 else xt_def[m - 2])
                    nc.vector.tensor_copy(
                        out=xt.rearrange("p (k c) -> p k c", k=4)[:, :, 0:rows],
                        in_=xp.rearrange("p (k c) -> p k c", k=4)[:, :, 0:rows])
                    # m 2..7: GEMM deferred into the early step tails
                    if m < 2:
                        gx_gemm(m, xt, p1ps, "pg1")

            # deferred m 6,7 GEMMs: deprioritized fillers for steps 0..3
            # (their gxT columns are not consumed until step 24)
            def_adds = []
            with tc.high_priority(offset=-600):
                for dm in (2, 3, 4, 5, 6, 7):
                    gx_gemm(dm, xt_def[dm - 2], spp, "pl0")

            # ---------- phase 2: recurrence with interleaved logits cols 0:V0 ----
            # super jobs: (sup, mt, k) matmuls + fin; emitted into PE stall windows
            s0_jobs = []
            for mt in range(NMT - 1):
                for sup in range(V0 // 512):
                    for k in range(4):
                        s0_jobs.append((sup, mt, k))
            s0_state = {"i": 0, "pl": None, "wl3": 0, "fins": []}

            def super0_next():
                sup, mt, k = s0_jobs[s0_state["i"]]
                s0_state["i"] += 1
                rows = min(128, M_TOK - 128 * mt)
                # deprioritized: fill PE/DVE idle slots, never delay the
                # recurrence chain (the scheduler would otherwise hoist these
                # between critical matmuls)
                with tc.high_priority(offset=-600):
                    if k == 0:
                        s0_state["pl"] = spp.tile([128, 512], f32, tag="pl0",
                                                  name=f"pl0_{sup}_{mt}")
                    pl = s0_state["pl"]
                    # 256-wide quanta (still 1 cyc/row in fp32r): halves the
                    # worst-case overrun past the moment h lands
                    for hh in (0, 256):
                        nc.tensor.matmul(
                            out=pl[0:rows, hh:hh + 256],
                            lhsT=hall_f[:, M_TOK * k + 128 * mt: M_TOK * k + 128 * mt + rows],
                            rhs=wl0[:, V0 * k + 512 * sup + hh: V0 * k + 512 * sup + hh + 256],
                            start=(k == 0 and hh == 0), stop=(k == 3 and hh == 256))
                    if k == 3:
                        stg = sst.tile([128, 512], f32, tag="stg0", name=f"stg0_{sup}_{mt}")
                        # half-width adds; made dependent on the emitting
                        # step's final h-write afterwards, so they can never
                        # slot in front of it on the in-order DVE queue
                        for hh in (0, 256):
                            fi = nc.vector.tensor_tensor(
                                out=stg[0:rows, hh:hh + 256], in0=pl[0:rows, hh:hh + 256],
                                in1=blin0[0:rows, 512 * sup + hh:512 * sup + hh + 256], op=ADD)
                            s0_state["fins"].append(fi)
                        nc.sync.dma_start(
                            out=out_d[128 * mt:128 * mt + rows, 512 * sup:512 * (sup + 1)],
                            in_=stg[0:rows, :])

            hall_b4 = hall_b.rearrange("p (k c) -> p k c", k=4)
            hall_f4 = hall_f.rearrange("p (k c) -> p k c", k=4)
            acto4 = acto.rearrange("p (q b) -> p q b", q=4)
            th4 = th_sb.rearrange("p (q b) -> p q b", q=4)
            # prefetch pieces for phase-3 supers 0,1 (weights + bias), SP queue
            wl3_ring = [p3w.tile([128, 4 * W3], f32r, tag="wl", name="wl0"),
                        p3w.tile([128, 4 * W3], f32r, tag="wl", name="wl1")]
            bl3_ring = [p3b.tile([128, W3], bf16, tag="bl3", name="bl3_0"),
                        p3b.tile([128, W3], bf16, tag="bl3", name="bl3_1")]
            pieces = (wl_pieces(0, wl3_ring[0]) + blin_pieces(0, bl3_ring[0])
                      + wl_pieces(1, wl3_ring[1]) + blin_pieces(1, bl3_ring[1]))
            piece_i = [0]

            with tc.tile_pool(name="rps", bufs=2, space="PSUM") as rps:
                for s in range(S):
                    # i and f share one psum bank (one group, one sig_if ACT)
                    pgif = rps.tile([128, 256], f32, tag="pgif", name=f"pgif_{s}")
                    pgg = rps.tile([128, 128], f32, tag="pgg", name=f"pgg_{s}")
                    pgo = rps.tile([128, 128], f32, tag="pgo", name=f"pgo_{s}")
                    tile_of = {0: pgif, 1: pgif, 2: pgg, 3: pgo}
                    col0 = {0: 0, 1: 128, 2: 0, 3: 0}
                    # whole-bank gxT injects: start each psum group.
                    # Independent of h -> run inside the previous step's tail.
                    for t in GATE_ORDER:
                        nc.tensor.matmul(
                            out=tile_of[t][:, col0[t]:col0[t] + 128],
                            lhsT=id128b[:],
                            rhs=gxT[t].rearrange("p (q c) -> p q c", q=4)[:, :, 32 * s:32 * (s + 1)],
                            start=(t != 1), stop=False)
                    # interleaved logits matmuls: also h(s)-independent tail filler
                    emitted = 0
                    while (emitted < 4 and s0_state["i"] < len(s0_jobs)
                           and s0_jobs[s0_state["i"]][1] < s // 4):
                        super0_next()
                        emitted += 1
                    # drip phase-3 prefetch pieces onto the SP queue
                    if s >= 4:
                        for _ in range(2):
                            if piece_i[0] < len(pieces):
                                dst, src = pieces[piece_i[0]]
                                piece_i[0] += 1
                                nc.sync.dma_start(out=dst, in_=src)
                    # W_hh @ h, order g, i, f, o; k outer so low-k matmuls can
                    # start off the first half of h
                    for t in GATE_ORDER:
                        for k in range(4):
                            for q in range(4):
                                if s == 0:
                                    rh = hT0[:, BL * k:BL * (k + 1)]
                                else:
                                    rh = hall_b[:, M_TOK * k + BL * (s - 1): M_TOK * k + BL * s]
                                nc.tensor.matmul(
                                    out=tile_of[t][:, col0[t] + 32 * q:col0[t] + 32 * (q + 1)],
                                    lhsT=whh_sb[:, G4 * k + 512 * t + 128 * q:
                                                G4 * k + 512 * t + 128 * (q + 1)],
                                    rhs=rh, start=False,
                                    stop=(q == 3 and k == 3 and t != 0))
                        if t == 2:
                            nc.scalar.activation(out=actg[:], in_=pgg[:], func=TANH)
                        elif t == 1:
                            nc.scalar.activation(out=actif[:], in_=pgif[:], func=SIG)
                    # cell update: t1 on Pool, t2 on DVE
                    nc.gpsimd.tensor_tensor(out=t1_sb[:], in0=actif[:, 0:128], in1=actg[:], op=MUL)
                    nc.vector.tensor_tensor(out=t2_sb[:], in0=actif[:, 128:256], in1=cT[:], op=MUL)
                    nc.vector.tensor_tensor(out=cT[:], in0=t1_sb[:], in1=t2_sb[:], op=ADD)
                    # th in halves: th_lo unblocks h_lo (k0-1) a hop earlier
                    nc.scalar.activation(out=th_sb[:, 0:64], in_=cT[:, 0:64], func=TANH)
                    nc.scalar.activation(out=th_sb[:, 64:128], in_=cT[:, 64:128], func=TANH)
                    nc.scalar.activation(out=acto[:], in_=pgo[:], func=SIG)
                    # h (bf16 first, in halves: k0-1 unblocks the next step's
                    # low-k matmuls while k2-3 is still being written)
                    nc.vector.tensor_tensor(out=hall_b4[:, 0:2, BL * s:BL * (s + 1)],
                                            in0=acto4[:, 0:2], in1=th4[:, 0:2], op=MUL)
                    hb_i = nc.vector.tensor_tensor(out=hall_b4[:, 2:4, BL * s:BL * (s + 1)],
                                                   in0=acto4[:, 2:4], in1=th4[:, 2:4], op=MUL)
                    nc.gpsimd.tensor_tensor(out=hall_f4[:, :, BL * s:BL * (s + 1)],
                                            in0=acto4[:], in1=th4[:], op=MUL)
                    import bass_rust as _br
                    _DI = _br.DependencyInfo(sync=True, no_sync=False)
                    for fi in s0_state["fins"]:
                        fi.ins.add_dependency(hb_i.ins.name, _DI)
                    s0_state["fins"] = []
                    for fi in def_adds[:2]:
                        fi.ins.add_dependency(hb_i.ins.name, _DI)
                    del def_adds[:2]
                # drain remaining super jobs and prefetch pieces
                while s0_state["i"] < len(s0_jobs):
                    super0_next()
                while piece_i[0] < len(pieces):
                    dst, src = pieces[piece_i[0]]
                    piece_i[0] += 1
                    nc.sync.dma_start(out=dst, in_=src)

            sst.release()
            spp.release()
            defp.release()
            gxp.release()

            # ---------- phase 3: logits cols V0:10000, fp32r ----------
            with tc.tile_pool(name="p3st", bufs=6) as p3st, \
                 tc.tile_pool(name="p3ps", bufs=2, space="PSUM") as p3ps:
                nst = 0
                # mt7's cols 0:V0 (its h finishes only at step 30) overlap the
                # first wl stream here instead of serializing after the loop
                mt, rows = NMT - 1, M_TOK - 128 * (NMT - 1)
                for sup in range(V0 // 512):
                    pl = p3ps.tile([128, W3], f32, tag="pl", name=f"pl7_{sup}")
                    for k in range(4):
                        nc.tensor.matmul(
                            out=pl[0:rows, 0:512],
                            lhsT=hall_f[:, M_TOK * k + 128 * mt: M_TOK * k + 128 * mt + rows],
                            rhs=wl0[:, V0 * k + 512 * sup: V0 * k + 512 * (sup + 1)],
                            start=(k == 0), stop=(k == 3))
                    stg = p3st.tile([128, W3], f32, tag="stg", name=f"stg7_{sup}")
                    nc.vector.tensor_tensor(out=stg[0:rows, 0:512], in0=pl[0:rows, 0:512],
                                            in1=blin0[0:rows, 512 * sup:512 * (sup + 1)], op=ADD)
                    eng = nc.sync if nst % 2 == 0 else nc.gpsimd
                    nst += 1
                    eng.dma_start(out=out_d[128 * mt:128 * mt + rows, 512 * sup:512 * (sup + 1)],
                                  in_=stg[0:rows, 0:512])
                for ns, (c0, c1) in enumerate(sup_bounds):
                    w_sup = c1 - c0
                    chunks = []
                    off = 0
                    while off < w_sup:
                        chunks.append((off, min(512, w_sup - off)))
                        off += 512
                    wl = wl3_ring[ns]
                    bl = bl3_ring[ns]
                    for m in range(NMT):
                        if m == 1 and ns + 2 < NS3:
                            wl3_ring.append(load_wl(ns + 2, nc.scalar))
                            bl3_ring.append(load_blin(ns + 2, nc.scalar))
                        rows = min(128, M_TOK - 128 * m)
                        pl = p3ps.tile([128, W3], f32, tag="pl")
                        for off, width in chunks:
                            for k in range(4):
                                nc.tensor.matmul(
                                    out=pl[0:rows, off:off + width],
                                    lhsT=hall_f[:, M_TOK * k + 128 * m: M_TOK * k + 128 * m + rows],
                                    rhs=wl[:, W3 * k + off: W3 * k + off + width],
                                    start=(k == 0), stop=(k == 3))
                        stg = p3st.tile([128, W3], f32, tag="stg")
                        nc.vector.tensor_tensor(out=stg[0:rows, 0:w_sup], in0=pl[0:rows, 0:w_sup],
                                                in1=bl[0:rows, 0:w_sup], op=ADD)
                        # rotate store queues (ACT joins once wl loads end)
                        if ns >= NS3 - 2:
                            eng = (nc.sync, nc.gpsimd, nc.scalar)[nst % 3]
                        else:
                            eng = nc.sync if nst % 2 == 0 else nc.gpsimd
                        nst += 1
                        eng.dma_start(out=out_d[128 * m:128 * m + rows, c0:c1],
                                      in_=stg[0:rows, 0:w_sup])

            wlp.release()
            p3b.release()
            p3w.release()

    nc.compile()
    return nc


def _prep_host(caps, latent, embed, W_ih, W_hh, b_ih, b_hh, W_lin, b_lin):
    import ml_dtypes
    bf = ml_dtypes.bfloat16
    caps = np.asarray(caps).astype(np.int32)
    latent = np.asarray(latent, dtype=np.float32)
    embed = np.ascontiguousarray(np.asarray(embed, dtype=np.float32).astype(bf))
    wihT = np.ascontiguousarray(np.asarray(W_ih, dtype=np.float32).T.astype(bf))  # [E, 4H]
    whhT = np.ascontiguousarray(np.asarray(W_hh, dtype=np.float32).T.astype(bf))  # [H, 4H]
    bias = (np.asarray(b_ih, dtype=np.float32) + np.asarray(b_hh, dtype=np.float32))
    # biasq[p, c] = bias[(c//128)*128 + p]
    blk = bias.reshape(G4 // 128, 128)            # [16, 128]
    biasq = np.ascontiguousarray(
        np.broadcast_to(blk.T[:, :, None], (128, G4 // 128, 128))
        .reshape(128, G4).astype(bf))
    wlinT = np.ascontiguousarray(np.asarray(W_lin, dtype=np.float32).T)   # [H, V]
    blinb = np.ascontiguousarray(np.broadcast_to(
        np.asarray(b_lin, dtype=np.float32)[None, :], (128, V)).astype(bf))
    id128 = np.eye(128, dtype=np.float32)
    id128b = np.eye(128).astype(bf)

    in_maps = []
    for c in range(NCORES):
        caps_sh = caps[c * BL:(c + 1) * BL]                   # [32, 32]
        tok_flat = caps_sh[:, :S].T.reshape(M_TOK)            # t-major [992]
        tok_pad = np.zeros(NMT * 128, dtype=np.int32)
        tok_pad[:M_TOK] = tok_flat
        tok = np.ascontiguousarray(tok_pad.reshape(NMT, 128).T)  # [128, NMT]
        in_maps.append(dict(
            emb=embed, wihT=wihT, whhT=whhT, biasq=biasq, wlinT=wlinT,
            blinb=blinb, tok=tok, lat=np.ascontiguousarray(latent[c * BL:(c + 1) * BL]),
            id128=id128, id128b=id128b,
        ))
    return in_maps


def kernel(caps, latent, embed, W_ih, W_hh, b_ih, b_hh, W_lin, b_lin):
    from concourse.bass_utils import run_bass_kernel_spmd

    if "nc" not in _CACHE:
        _CACHE["nc"] = _build()
    nc = _CACHE["nc"]

    in_maps = _prep_host(caps, latent, embed, W_ih, W_hh, b_ih, b_hh, W_lin, b_lin)
    res = run_bass_kernel_spmd(nc, in_maps, core_ids=list(range(NCORES)))
    out = np.zeros((T, B_FULL, V), dtype=np.float32)
    for c in range(NCORES):
        shard = res.results[c]["out"].reshape(S, BL, V)
        out[1:, c * BL:(c + 1) * BL, :] = shard
    return out


# revision 91
# speedup vs baseline: 2.5624x; 1.0002x over previous
"""Teacher-forced decoder LSTM on 8 TRN2 NeuronCores.

Problem: B=256, T=32, V=10000, E=H=512 (fp32).
  step s in 0..30: x = embed[caps[:, s]]
                   gates = x@W_ih.T + h@W_hh.T + b     (i,f,g,o)
                   c = sig(f)*c + sig(i)*tanh(g); h = sig(o)*tanh(c)
                   out[s+1] = h@W_lin.T + b_lin
  out[0] = 0.  Output [T, B, V].

Sharding: data-parallel over batch, B_local=32 per core.

Layout: the recurrence runs fully TRANSPOSED (gate/hidden dims on
partitions, batch on the free axis) so each recurrent matmul moves only
32 columns. bf16 weights/activations in the gate path give 1 cyc/row on
the PE at any free size; the logits GEMM stays fp32r off the f32 copy of
h for accuracy.

  phase 1: gather X = embed[tok], PE-transpose, GxT = W_ih@X.T + b as
     bf16 tiles [128, (q)(tok)] per gate type (bias folded in).
  phase 2 (recurrent): per step 4 whole-bank gxT-inject matmuls (start
     the psum group) + 64 W_hh matmuls, all [*, 32/128]-moving bf16; ACT
     sig/tanh straight from PSUM; DVE/Pool cell update; h written
     directly into transposed history (bf16 for the recurrence, f32r for
     the logits GEMM) - no per-step transposes. Logits cols 0:1024 are
     interleaved into the PE stall windows between steps, and the first
     two phase-3 weight super-chunks prefetch on the idle SP DMA queue.
  phase 3: logits cols 1024:10000 as fp32r GEMM streamed per ~1800-col
     super-chunk, stores alternating SP/Pool DMA queues.
"""
import numpy as np

B_FULL, T, V, E, H = 256, 32, 10000, 512, 512
NCORES = 8
BL = B_FULL // NCORES          # 32 batch per core
S = T - 1                      # 31 recurrent steps
M_TOK = S * BL                 # 992 token rows per core
G4 = 4 * H                     # 2048 gate dims
NMT = (M_TOK + 127) // 128     # 8 token m-tiles (last is 96 rows)
V0 = 2048                      # vocab cols done inside phase 2
W3 = 1536                      # phase-3 super-chunk width (3 x 512)
NS3 = 6                        # phase-3 super count (5 x 1536 + 272)

_CACHE = {}


def _build():
    import concourse.bacc as bacc
    import concourse.mybir as mybir
    from concourse.tile import TileContext
    import concourse.bass as bass

    f32 = mybir.dt.float32
    f32r = mybir.dt.float32r
    bf16 = mybir.dt.bfloat16
    i32 = mybir.dt.int32
    SIG = mybir.ActivationFunctionType.Sigmoid
    TANH = mybir.ActivationFunctionType.Tanh
    ADD = mybir.AluOpType.add
    MUL = mybir.AluOpType.mult

    nc = bacc.Bacc()

    emb_d = nc.dram_tensor("emb", [V, E], bf16, kind="ExternalInput")
    wihT_d = nc.dram_tensor("wihT", [E, G4], bf16, kind="ExternalInput")
    whhT_d = nc.dram_tensor("whhT", [H, G4], bf16, kind="ExternalInput")
    biasq_d = nc.dram_tensor("biasq", [128, G4], bf16, kind="ExternalInput")
    wlinT_d = nc.dram_tensor("wlinT", [H, V], f32r, kind="ExternalInput")
    blinb_d = nc.dram_tensor("blinb", [128, V], bf16, kind="ExternalInput")
    tok_d = nc.dram_tensor("tok", [128, NMT], i32, kind="ExternalInput")
    lat_d = nc.dram_tensor("lat", [BL, H], f32r, kind="ExternalInput")
    id128_d = nc.dram_tensor("id128", [128, 128], f32r, kind="ExternalInput")
    id128b_d = nc.dram_tensor("id128b", [128, 128], bf16, kind="ExternalInput")
    out_d = nc.dram_tensor("out", [M_TOK, V], f32, kind="ExternalOutput")

    GATE_ORDER = (2, 0, 1, 3)   # g, i, f, o: start the tanh_g chain early

    with TileContext(nc) as tc:
        with tc.tile_pool(name="const", bufs=1) as cp, \
             tc.tile_pool(name="state", bufs=1) as st:

            # ---------- constants ----------
            tok_sb = cp.tile([128, NMT], i32, tag="tok_sb")
            nc.sync.dma_start(out=tok_sb[:], in_=tok_d[:])
            id128 = cp.tile([128, 128], f32r, tag="id128")
            nc.sync.dma_start(out=id128[:], in_=id128_d[:])
            lat_sb = cp.tile([BL, H], f32r, tag="lat_sb")
            nc.sync.dma_start(out=lat_sb[:], in_=lat_d[:])
            id128b = cp.tile([128, 128], bf16, tag="id128b")
            nc.sync.dma_start(out=id128b[:], in_=id128b_d[:])

            # ---------- state ----------
            hall_f = st.tile([128, 4 * M_TOK], f32r, tag="hall_f")
            hall_b = st.tile([128, 4 * M_TOK], bf16, tag="hall_b")
            hT0 = st.tile([128, 4 * BL], bf16, tag="hT0")   # col = 32k + b
            cT = st.tile([128, 4 * BL], f32, tag="cT")
            nc.vector.memset(cT[:], 0.0)
            actif = st.tile([128, 8 * BL], f32, tag="actif")
            actg = st.tile([128, 4 * BL], f32, tag="actg")
            acto = st.tile([128, 4 * BL], f32, tag="acto")
            t1_sb = st.tile([128, 4 * BL], f32, tag="t1_sb")
            t2_sb = st.tile([128, 4 * BL], f32, tag="t2_sb")
            th_sb = st.tile([128, 4 * BL], f32, tag="th_sb")
            # preload sigmoid/tanh activation tables during setup
            nc.scalar.activation(out=t1_sb[0:1, 0:1], in_=cT[0:1, 0:1], func=SIG)
            nc.scalar.activation(out=t2_sb[0:1, 0:1], in_=cT[0:1, 0:1], func=TANH)

            # phase-3 weight/bias rings (allocated first so they outlive gxp)
            p3w = tc.alloc_tile_pool(name="p3w", bufs=2)
            p3b = tc.alloc_tile_pool(name="p3b", bufs=3)
            sup_bounds = [(V0 + W3 * i, min(V0 + W3 * (i + 1), V)) for i in range(NS3)]

            def load_wl(ns, eng):
                c0, c1 = sup_bounds[ns]
                wl = p3w.tile([128, 4 * W3], f32r, tag="wl", name=f"wl{ns}")
                for k in range(4):
                    eng.dma_start(out=wl[:, W3 * k:W3 * k + (c1 - c0)],
                                  in_=wlinT_d[128 * k:128 * (k + 1), c0:c1])
                return wl

            def load_blin(ns, eng):
                c0, c1 = sup_bounds[ns]
                bl = p3b.tile([128, W3], bf16, tag="bl3", name=f"bl3_{ns}")
                eng.dma_start(out=bl[:, 0:c1 - c0], in_=blinb_d[:, c0:c1])
                return bl

            def wl_pieces(ns, wl):
                # split one wl super-load into small DMAs that drip onto the
                # SP queue between recurrence steps (big transfers would
                # block the interleaved logit stores behind them)
                c0, c1 = sup_bounds[ns]
                ps = []
                for k in range(4):
                    for h0 in range(0, c1 - c0, 512):
                        h1 = min(h0 + 512, c1 - c0)
                        ps.append((wl[:, W3 * k + h0:W3 * k + h1],
                                   wlinT_d[128 * k:128 * (k + 1), c0 + h0:c0 + h1]))
                return ps

            def blin_pieces(ns, bl):
                c0, c1 = sup_bounds[ns]
                ps = []
                for h0 in range(0, c1 - c0, 512):
                    h1 = min(h0 + 512, c1 - c0)
                    ps.append((bl[:, h0:h1], blinb_d[:, c0 + h0:c0 + h1]))
                return ps

            # super0 weights/bias live through phase 3 (mt7 jobs run there)
            wlp = tc.alloc_tile_pool(name="wlp", bufs=1)
            # long-lived recurrence inputs (released before phase 3)
            gxp = tc.alloc_tile_pool(name="gxp", bufs=1)
            whh_sb = gxp.tile([128, 4 * G4], bf16, tag="whh_sb")
            gxT = [gxp.tile([128, 4 * M_TOK], bf16, tag=f"gxT{t}", name=f"gxT{t}")
                   for t in range(4)]
            # super0 (vocab cols 0:V0) weights, interleaved into phase 2
            wl0 = wlp.tile([128, 4 * V0], f32r, tag="wl0")
            blin0 = wlp.tile([128, V0], bf16, tag="blin0")
            # tensors used by deferred phase-1 m-tiles (6,7), whose GEMMs run
            # inside the empty tails of recurrence steps 0..3
            defp = tc.alloc_tile_pool(name="defp", bufs=1)
            wih_sb = defp.tile([128, 4 * G4], bf16, tag="wih_sb")
            biasq = defp.tile([128, G4], bf16, tag="biasq")
            xt_def = [defp.tile([128, 512], bf16, tag=f"xtd{m}", name=f"xtd{m}")
                      for m in (2, 3, 4, 5, 6, 7)]

            spp = tc.alloc_tile_pool(name="spp", bufs=2, space="PSUM")
            sst = tc.alloc_tile_pool(name="sst", bufs=3)

            # ---------- phase 1: gather X, transpose, GxT = W_ih@X.T + b ----------
            with tc.tile_pool(name="p1", bufs=1) as p1, \
                 tc.tile_pool(name="p1ps", bufs=2, space="PSUM") as p1ps, \
                 tc.tile_pool(name="xps", bufs=2, space="PSUM") as xps, \
                 tc.tile_pool(name="tpsum", bufs=1, space="PSUM") as tps:
                # W_ih in 4 k-chunk DMAs so the first GEMM starts early;
                # wl0 after it on the same queue (not needed until step 4)
                for k in range(4):
                    eng = nc.scalar if k % 2 == 0 else nc.sync
                    eng.dma_start(out=wih_sb[:, G4 * k:G4 * (k + 1)],
                                  in_=wihT_d[128 * k:128 * (k + 1), :])
                nc.sync.dma_start(out=biasq[:], in_=biasq_d[:])
                # whh in per-k chunks after wih+biasq: step 0's low-k
                # matmuls can start as soon as their chunk lands
                for k in range(4):
                    nc.sync.dma_start(out=whh_sb[:, G4 * k:G4 * (k + 1)],
                                      in_=whhT_d[128 * k:128 * (k + 1), :])
                nc.sync.dma_start(out=blin0[:], in_=blinb_d[:, 0:V0])
                for k in range(4):
                    nc.scalar.dma_start(out=wl0[:, V0 * k:V0 * (k + 1)],
                                        in_=wlinT_d[128 * k:128 * (k + 1), 0:V0])

                # transpose h0 = latent -> hT0 (one psum tile, one copy)
                pt0 = tps.tile([128, 128], f32r, tag="pt0")
                for k in range(4):
                    nc.tensor.transpose(out=pt0[0:128, BL * k:BL * (k + 1)],
                                        in_=lat_sb[:, 128 * k:128 * (k + 1)],
                                        identity=id128[0:BL, 0:BL])
                nc.vector.tensor_copy(out=hT0[:], in_=pt0[:])

                def gx_gemm(m, xt, pool, tag):
                    rows = min(128, M_TOK - 128 * m)
                    for t in range(4):
                        pg = pool.tile([128, 512], f32, tag=tag, name=f"pg1_{m}_{t}")
                        for q in range(4):
                            for k in range(4):
                                nc.tensor.matmul(
                                    out=pg[:, 128 * q:128 * q + rows],
                                    lhsT=wih_sb[:, G4 * k + 512 * t + 128 * q:
                                                G4 * k + 512 * t + 128 * (q + 1)],
                                    rhs=xt[:, 128 * k:128 * k + rows],
                                    start=(k == 0), stop=(k == 3))
                        # add bias, cast to bf16 (DVE: GPSIMD cannot read PSUM)
                        ai = nc.vector.tensor_tensor(
                            out=gxT[t].rearrange("p (q c) -> p q c", q=4)[:, :, 128 * m:128 * m + rows],
                            in0=pg.rearrange("p (q c) -> p q c", q=4)[:, :, 0:rows],
                            in1=biasq[:, 512 * t:512 * (t + 1)].rearrange(
                                "p (q c) -> p q c", q=4)[:, :, 0:rows],
                            op=ADD)
                        if m >= 2:
                            def_adds.append(ai)

                for m in range(NMT):
                    rows = min(128, M_TOK - 128 * m)
                    x_m = p1.tile([128, E], bf16, tag="x_m", bufs=2, name=f"x_m{m}")
                    nc.gpsimd.indirect_dma_start(
                        out=x_m[0:rows, :], out_offset=None, in_=emb_d[:],
                        in_offset=bass.IndirectOffsetOnAxis(ap=tok_sb[0:rows, m:m + 1], axis=0))
                    xp = xps.tile([128, 512], bf16, tag="xp", name=f"xp{m}")
                    for k in range(4):
                        nc.tensor.transpose(out=xp[0:128, 128 * k:128 * k + rows],
                                            in_=x_m[0:rows, 128 * k:128 * (k + 1)],
                                            identity=id128b[0:rows, 0:rows])
                    xt = (p1.tile([128, 512], bf16, tag="xt", bufs=2, name=f"xt{m}")
                          if m < 2 else xt_def[m - 2])
                    nc.vector.tensor_copy(
                        out=xt.rearrange("p (k c) -> p k c", k=4)[:, :, 0:rows],
                        in_=xp.rearrange("p (k c) -> p k c", k=4)[:, :, 0:rows])
                    # m 2..7: GEMM deferred into the early step tails
                    if m < 2:
                        gx_gemm(m, xt, p1ps, "pg1")

            # deferred m 6,7 GEMMs: deprioritized fillers for steps 0..3
            # (their gxT columns are not consumed until step 24)
            def_adds = []
            with tc.high_priority(offset=-600):
                for dm in (2, 3, 4, 5, 6, 7):
                    gx_gemm(dm, xt_def[dm - 2], spp, "pl0")

            # ---------- phase 2: recurrence with interleaved logits cols 0:V0 ----
            # super jobs: (sup, mt, k) matmuls + fin; emitted into PE stall windows
            s0_jobs = []
            for mt in range(NMT - 1):
                for sup in range(V0 // 512):
                    for k in range(4):
                        s0_jobs.append((sup, mt, k))
            s0_state = {"i": 0, "pl": None, "wl3": 0, "fins": []}

            def super0_next():
                sup, mt, k = s0_jobs[s0_state["i"]]
                s0_state["i"] += 1
                rows = min(128, M_TOK - 128 * mt)
                # deprioritized: fill PE/DVE idle slots, never delay the
                # recurrence chain (the scheduler would otherwise hoist these
                # between critical matmuls)
                with tc.high_priority(offset=-600):
                    if k == 0:
                        s0_state["pl"] = spp.tile([128, 512], f32, tag="pl0",
                                                  name=f"pl0_{sup}_{mt}")
                    pl = s0_state["pl"]
                    # 256-wide quanta (still 1 cyc/row in fp32r): halves the
                    # worst-case overrun past the moment h lands
                    for hh in (0, 256):
                        nc.tensor.matmul(
                            out=pl[0:rows, hh:hh + 256],
                            lhsT=hall_f[:, M_TOK * k + 128 * mt: M_TOK * k + 128 * mt + rows],
                            rhs=wl0[:, V0 * k + 512 * sup + hh: V0 * k + 512 * sup + hh + 256],
                            start=(k == 0 and hh == 0), stop=(k == 3 and hh == 256))
                    if k == 3:
                        stg = sst.tile([128, 512], f32, tag="stg0", name=f"stg0_{sup}_{mt}")
                        # half-width adds; made dependent on the emitting
                        # step's final h-write afterwards, so they can never
                        # slot in front of it on the in-order DVE queue
                        for hh in (0, 256):
                            fi = nc.vector.tensor_tensor(
                                out=stg[0:rows, hh:hh + 256], in0=pl[0:rows, hh:hh + 256],
                                in1=blin0[0:rows, 512 * sup + hh:512 * sup + hh + 256], op=ADD)
                            s0_state["fins"].append(fi)
                        nc.sync.dma_start(
                            out=out_d[128 * mt:128 * mt + rows, 512 * sup:512 * (sup + 1)],
                            in_=stg[0:rows, :])

            hall_b4 = hall_b.rearrange("p (k c) -> p k c", k=4)
            hall_f4 = hall_f.rearrange("p (k c) -> p k c", k=4)
            acto4 = acto.rearrange("p (q b) -> p q b", q=4)
            th4 = th_sb.rearrange("p (q b) -> p q b", q=4)
            # prefetch pieces for phase-3 supers 0,1 (weights + bias), SP queue
            wl3_ring = [p3w.tile([128, 4 * W3], f32r, tag="wl", name="wl0"),
                        p3w.tile([128, 4 * W3], f32r, tag="wl", name="wl1")]
            bl3_ring = [p3b.tile([128, W3], bf16, tag="bl3", name="bl3_0"),
                        p3b.tile([128, W3], bf16, tag="bl3", name="bl3_1")]
            pieces = (wl_pieces(0, wl3_ring[0]) + blin_pieces(0, bl3_ring[0])
                      + wl_pieces(1, wl3_ring[1]) + blin_pieces(1, bl3_ring[1]))
            piece_i = [0]

            with tc.tile_pool(name="rps", bufs=2, space="PSUM") as rps:
                for s in range(S):
                    # i and f share one psum bank (one group, one sig_if ACT)
                    pgif = rps.tile([128, 256], f32, tag="pgif", name=f"pgif_{s}")
                    pgg = rps.tile([128, 128], f32, tag="pgg", name=f"pgg_{s}")
                    pgo = rps.tile([128, 128], f32, tag="pgo", name=f"pgo_{s}")
                    tile_of = {0: pgif, 1: pgif, 2: pgg, 3: pgo}
                    col0 = {0: 0, 1: 128, 2: 0, 3: 0}
                    # whole-bank gxT injects: start each psum group.
                    # Independent of h -> run inside the previous step's tail.
                    for t in GATE_ORDER:
                        nc.tensor.matmul(
                            out=tile_of[t][:, col0[t]:col0[t] + 128],
                            lhsT=id128b[:],
                            rhs=gxT[t].rearrange("p (q c) -> p q c", q=4)[:, :, 32 * s:32 * (s + 1)],
                            start=(t != 1), stop=False)
                    # interleaved logits matmuls: also h(s)-independent tail filler
                    emitted = 0
                    while (emitted < 4 and s0_state["i"] < len(s0_jobs)
                           and s0_jobs[s0_state["i"]][1] < s // 4):
                        super0_next()
                        emitted += 1
                    # drip phase-3 prefetch pieces onto the SP queue
                    if s >= 4:
                        for _ in range(3):
                            if piece_i[0] < len(pieces):
                                dst, src = pieces[piece_i[0]]
                                piece_i[0] += 1
                                nc.sync.dma_start(out=dst, in_=src)
                    # W_hh @ h, order g, i, f, o; k outer so low-k matmuls can
                    # start off the first half of h
                    for t in GATE_ORDER:
                        for k in range(4):
                            for q in range(4):
                                if s == 0:
                                    rh = hT0[:, BL * k:BL * (k + 1)]
                                else:
                                    rh = hall_b[:, M_TOK * k + BL * (s - 1): M_TOK * k + BL * s]
                                nc.tensor.matmul(
                                    out=tile_of[t][:, col0[t] + 32 * q:col0[t] + 32 * (q + 1)],
                                    lhsT=whh_sb[:, G4 * k + 512 * t + 128 * q:
                                                G4 * k + 512 * t + 128 * (q + 1)],
                                    rhs=rh, start=False,
                                    stop=(q == 3 and k == 3 and t != 0))
                        if t == 2:
                            nc.scalar.activation(out=actg[:], in_=pgg[:], func=TANH)
                        elif t == 1:
                            nc.scalar.activation(out=actif[:], in_=pgif[:], func=SIG)
                    # cell update: t1 on Pool, t2 on DVE
                    nc.gpsimd.tensor_tensor(out=t1_sb[:], in0=actif[:, 0:128], in1=actg[:], op=MUL)
                    nc.vector.tensor_tensor(out=t2_sb[:], in0=actif[:, 128:256], in1=cT[:], op=MUL)
                    nc.vector.tensor_tensor(out=cT[:], in0=t1_sb[:], in1=t2_sb[:], op=ADD)
                    # th in halves: th_lo unblocks h_lo (k0-1) a hop earlier
                    nc.scalar.activation(out=th_sb[:, 0:64], in_=cT[:, 0:64], func=TANH)
                    nc.scalar.activation(out=th_sb[:, 64:128], in_=cT[:, 64:128], func=TANH)
                    nc.scalar.activation(out=acto[:], in_=pgo[:], func=SIG)
                    # h (bf16 first, in halves: k0-1 unblocks the next step's
                    # low-k matmuls while k2-3 is still being written)
                    nc.vector.tensor_tensor(out=hall_b4[:, 0:2, BL * s:BL * (s + 1)],
                                            in0=acto4[:, 0:2], in1=th4[:, 0:2], op=MUL)
                    hb_i = nc.vector.tensor_tensor(out=hall_b4[:, 2:4, BL * s:BL * (s + 1)],
                                                   in0=acto4[:, 2:4], in1=th4[:, 2:4], op=MUL)
                    nc.gpsimd.tensor_tensor(out=hall_f4[:, :, BL * s:BL * (s + 1)],
                                            in0=acto4[:], in1=th4[:], op=MUL)
                    import bass_rust as _br
                    _DI = _br.DependencyInfo(sync=True, no_sync=False)
                    for fi in s0_state["fins"]:
                        fi.ins.add_dependency(hb_i.ins.name, _DI)
                    s0_state["fins"] = []
                    for fi in def_adds[:2]:
                        fi.ins.add_dependency(hb_i.ins.name, _DI)
                    del def_adds[:2]
                # drain remaining super jobs and prefetch pieces
                while s0_state["i"] < len(s0_jobs):
                    super0_next()
                while piece_i[0] < len(pieces):
                    dst, src = pieces[piece_i[0]]
                    piece_i[0] += 1
                    nc.sync.dma_start(out=dst, in_=src)

            sst.release()
            spp.release()
            defp.release()
            gxp.release()

            # ---------- phase 3: logits cols V0:10000, fp32r ----------
            with tc.tile_pool(name="p3st", bufs=6) as p3st, \
                 tc.tile_pool(name="p3ps", bufs=2, space="PSUM") as p3ps:
                nst = 0
                # mt7's cols 0:V0 (its h finishes only at step 30) overlap the
                # first wl stream here instead of serializing after the loop
                mt, rows = NMT - 1, M_TOK - 128 * (NMT - 1)
                for sup in range(V0 // 512):
                    pl = p3ps.tile([128, W3], f32, tag="pl", name=f"pl7_{sup}")
                    for k in range(4):
                        nc.tensor.matmul(
                            out=pl[0:rows, 0:512],
                            lhsT=hall_f[:, M_TOK * k + 128 * mt: M_TOK * k + 128 * mt + rows],
                            rhs=wl0[:, V0 * k + 512 * sup: V0 * k + 512 * (sup + 1)],
                            start=(k == 0), stop=(k == 3))
                    stg = p3st.tile([128, W3], f32, tag="stg", name=f"stg7_{sup}")
                    nc.vector.tensor_tensor(out=stg[0:rows, 0:512], in0=pl[0:rows, 0:512],
                                            in1=blin0[0:rows, 512 * sup:512 * (sup + 1)], op=ADD)
                    eng = nc.sync if nst % 2 == 0 else nc.gpsimd
                    nst += 1
                    eng.dma_start(out=out_d[128 * mt:128 * mt + rows, 512 * sup:512 * (sup + 1)],
                                  in_=stg[0:rows, 0:512])
                for ns, (c0, c1) in enumerate(sup_bounds):
                    w_sup = c1 - c0
                    chunks = []
                    off = 0
                    while off < w_sup:
                        chunks.append((off, min(512, w_sup - off)))
                        off += 512
                    wl = wl3_ring[ns]
                    bl = bl3_ring[ns]
                    for m in range(NMT):
                        if m == 1 and ns + 2 < NS3:
                            wl3_ring.append(load_wl(ns + 2, nc.scalar))
                            bl3_ring.append(load_blin(ns + 2, nc.scalar))
                        rows = min(128, M_TOK - 128 * m)
                        pl = p3ps.tile([128, W3], f32, tag="pl")
                        for off, width in chunks:
                            for k in range(4):
                                nc.tensor.matmul(
                                    out=pl[0:rows, off:off + width],
                                    lhsT=hall_f[:, M_TOK * k + 128 * m: M_TOK * k + 128 * m + rows],
                                    rhs=wl[:, W3 * k + off: W3 * k + off + width],
                                    start=(k == 0), stop=(k == 3))
                        stg = p3st.tile([128, W3], f32, tag="stg")
                        nc.vector.tensor_tensor(out=stg[0:rows, 0:w_sup], in0=pl[0:rows, 0:w_sup],
                                                in1=bl[0:rows, 0:w_sup], op=ADD)
                        # rotate store queues (ACT joins once wl loads end)
                        if ns >= NS3 - 2:
                            eng = (nc.sync, nc.gpsimd, nc.scalar)[nst % 3]
                        else:
                            eng = nc.sync if nst % 2 == 0 else nc.gpsimd
                        nst += 1
                        eng.dma_start(out=out_d[128 * m:128 * m + rows, c0:c1],
                                      in_=stg[0:rows, 0:w_sup])

            wlp.release()
            p3b.release()
            p3w.release()

    nc.compile()
    return nc


def _prep_host(caps, latent, embed, W_ih, W_hh, b_ih, b_hh, W_lin, b_lin):
    import ml_dtypes
    bf = ml_dtypes.bfloat16
    caps = np.asarray(caps).astype(np.int32)
    latent = np.asarray(latent, dtype=np.float32)
    embed = np.ascontiguousarray(np.asarray(embed, dtype=np.float32).astype(bf))
    wihT = np.ascontiguousarray(np.asarray(W_ih, dtype=np.float32).T.astype(bf))  # [E, 4H]
    whhT = np.ascontiguousarray(np.asarray(W_hh, dtype=np.float32).T.astype(bf))  # [H, 4H]
    bias = (np.asarray(b_ih, dtype=np.float32) + np.asarray(b_hh, dtype=np.float32))
    # biasq[p, c] = bias[(c//128)*128 + p]
    blk = bias.reshape(G4 // 128, 128)            # [16, 128]
    biasq = np.ascontiguousarray(
        np.broadcast_to(blk.T[:, :, None], (128, G4 // 128, 128))
        .reshape(128, G4).astype(bf))
    wlinT = np.ascontiguousarray(np.asarray(W_lin, dtype=np.float32).T)   # [H, V]
    blinb = np.ascontiguousarray(np.broadcast_to(
        np.asarray(b_lin, dtype=np.float32)[None, :], (128, V)).astype(bf))
    id128 = np.eye(128, dtype=np.float32)
    id128b = np.eye(128).astype(bf)

    in_maps = []
    for c in range(NCORES):
        caps_sh = caps[c * BL:(c + 1) * BL]                   # [32, 32]
        tok_flat = caps_sh[:, :S].T.reshape(M_TOK)            # t-major [992]
        tok_pad = np.zeros(NMT * 128, dtype=np.int32)
        tok_pad[:M_TOK] = tok_flat
        tok = np.ascontiguousarray(tok_pad.reshape(NMT, 128).T)  # [128, NMT]
        in_maps.append(dict(
            emb=embed, wihT=wihT, whhT=whhT, biasq=biasq, wlinT=wlinT,
            blinb=blinb, tok=tok, lat=np.ascontiguousarray(latent[c * BL:(c + 1) * BL]),
            id128=id128, id128b=id128b,
        ))
    return in_maps


def kernel(caps, latent, embed, W_ih, W_hh, b_ih, b_hh, W_lin, b_lin):
    from concourse.bass_utils import run_bass_kernel_spmd

    if "nc" not in _CACHE:
        _CACHE["nc"] = _build()
    nc = _CACHE["nc"]

    in_maps = _prep_host(caps, latent, embed, W_ih, W_hh, b_ih, b_hh, W_lin, b_lin)
    res = run_bass_kernel_spmd(nc, in_maps, core_ids=list(range(NCORES)))
    out = np.zeros((T, B_FULL, V), dtype=np.float32)
    for c in range(NCORES):
        shard = res.results[c]["out"].reshape(S, BL, V)
        out[1:, c * BL:(c + 1) * BL, :] = shard
    return out
